# revision 1
# baseline (speedup 1.0000x reference)
"""Trainium2 Bass kernel for a dense transformer block, distributed over 8
NeuronCores.

Sharding:
  phase 1 (attention): tensor-parallel over heads — each core computes 2 of
    the 16 heads end-to-end (QKV projections + causal softmax(QK^T)V), and
    returns the unnormalized per-head output O^T together with the softmax
    denominators (obtained via a ones-column appended to V).
  phase 2 (Wo + norms + FFN): data-parallel over tokens — each core handles
    512 of the 4096 token rows with replicated weights.

The host glues the phases: transposes x, normalizes/concats heads, and
re-shards tokens.  All matmuls run as float32r (full-rate fp32 PE mode).
"""

import math
from contextlib import ExitStack

import ml_dtypes
import numpy as np

BF_NP = ml_dtypes.bfloat16

import concourse.bass as bass
import concourse.mybir as mybir
import concourse.tile as tile
from concourse import bacc
from concourse.bass_utils import run_bass_kernel_spmd
from concourse.masks import make_identity, make_upper_triangular

FP = mybir.dt.float32
FPR = mybir.dt.float32r
BF = mybir.dt.bfloat16
AF = mybir.ActivationFunctionType

N_CORES = 8
P = 128
EPS = 1e-6

# exec times (ns) of the most recent kernel() call, one entry per phase, when
# tracing was enabled via BASS_TRACE=1; None entries otherwise.
LAST_EXEC_NS = []


def _fpr(ap):
    return ap.bitcast(FPR)


# --------------------------------------------------------------------------
# phase 1: per-core attention over a pair of heads
# --------------------------------------------------------------------------

def build_phase1(B, T, C, DH):
    HP = 2                      # heads per core
    DA = DH + 1                 # head dim + ones row (softmax denominator)
    NCC = C // P                # contraction chunks
    NT = T // P                 # key/value blocks of 128
    NQ = T // 512               # query chunks of 512
    NK = T // 1024              # query tiles of 1024
    scale = float(C) ** -0.5    # NOTE: reference scales by C**-0.5, not DH

    nc = bacc.Bacc("TRN2", debug=False)
    xT_d = nc.dram_tensor("xT", [B, C, T], BF, kind="ExternalInput").ap()
    wq_d = nc.dram_tensor("wq", [C, HP * DH], BF, kind="ExternalInput").ap()
    wk_d = nc.dram_tensor("wk", [C, HP * DH], BF, kind="ExternalInput").ap()
    wv_d = nc.dram_tensor("wv", [C, HP * DH], BF, kind="ExternalInput").ap()
    ot_d = nc.dram_tensor("ot", [B, HP, DA, T], FP, kind="ExternalOutput").ap()

    with tile.TileContext(nc) as tc, ExitStack() as ctx:
        const = ctx.enter_context(tc.tile_pool(name="const", bufs=1))
        xpool = ctx.enter_context(tc.tile_pool(name="xp", bufs=1))
        wpool = ctx.enter_context(tc.tile_pool(name="wp", bufs=1))
        qk_pool = ctx.enter_context(tc.tile_pool(name="qk", bufs=2))
        vt_pool = ctx.enter_context(tc.tile_pool(name="vtp", bufs=2))
        vaug_pool = ctx.enter_context(tc.tile_pool(name="vaug", bufs=2))
        pt_pool = ctx.enter_context(tc.tile_pool(name="pt", bufs=4))
        ot_pool = ctx.enter_context(tc.tile_pool(name="otp", bufs=2))

        # additive mask for the diagonal 128x128 block of S^T [s', q']:
        # 0 where q' >= s' (causal-valid), -1e30 where q' < s'
        negmask = const.tile([P, P], FP)
        nc.gpsimd.memset(negmask[:], 0.0)
        nc.gpsimd.affine_select(
            out=negmask[:], in_=negmask[:],
            compare_op=mybir.AluOpType.is_ge, fill=-1e30,
            base=0, pattern=[[1, P]], channel_multiplier=-1)
        ident = const.tile([P, P], BF)
        make_identity(nc, ident[:])
        ones_col = const.tile([P, NT * HP, 1], FP)
        nc.vector.memset(ones_col[:], 1.0)

        # weight chunks, loaded once
        wts = {}
        for name, src in (("q", wq_d), ("k", wk_d), ("v", wv_d)):
            wts[name] = []
            for c in range(NCC):
                t = wpool.tile([P, HP * DH], BF, tag=f"w{name}{c}")
                nc.sync.dma_start(out=t[:], in_=src[c * P:(c + 1) * P, :])
                wts[name].append(t)

        for b in range(B):
            xts = []
            for c in range(NCC):
                xt = xpool.tile([P, T], BF, tag=f"x{c}")
                nc.sync.dma_start(out=xt[:], in_=xT_d[b, c * P:(c + 1) * P, :])
                xts.append(xt)

            qt = qk_pool.tile([P, T], BF, tag="qt")
            kt = qk_pool.tile([P, T], BF, tag="kt")
            vaug = vaug_pool.tile([P, NT * HP, DA], BF, tag="vaug")
            # ones column per head-block (softmax denominator row of O^T)
            nc.vector.tensor_copy(vaug[:, :, DA - 1:DA], ones_col[:])

            with tc.tile_pool(name="proj_ps", bufs=3, space="PSUM") as proj_ps, \
                 tc.tile_pool(name="vt_ps", bufs=2, space="PSUM") as vt_ps:
                for wt, dst in ((wts["q"], qt), (wts["k"], kt)):
                    for n in range(NQ):
                        ps = proj_ps.tile([P, 512], FP, tag="proj")
                        for c in range(NCC):
                            nc.tensor.matmul(
                                ps[:], wt[c][:], xts[c][:, n * 512:(n + 1) * 512],
                                start=(c == 0), stop=(c == NCC - 1))
                        nc.vector.tensor_copy(dst[:, n * 512:(n + 1) * 512], ps[:])
                # V, then transpose into [s, d] layout with ones columns
                for n in range(NQ):
                    ps = proj_ps.tile([P, 512], FP, tag="proj")
                    for c in range(NCC):
                        nc.tensor.matmul(
                            ps[:], wts["v"][c][:], xts[c][:, n * 512:(n + 1) * 512],
                            start=(c == 0), stop=(c == NCC - 1))
                    vt = vt_pool.tile([P, 512], BF, tag="vt")
                    nc.vector.tensor_copy(vt[:], ps[:])
                    for u in range(4):
                        j = 4 * n + u
                        tp = vt_ps.tile([P, P], BF, tag="vtp")
                        nc.tensor.transpose(tp[:], vt[:, u * P:(u + 1) * P], ident[:])
                        nc.vector.tensor_copy(
                            vaug[:, j * HP, 0:DH], tp[:, 0:DH])
                        nc.vector.tensor_copy(
                            vaug[:, j * HP + 1, 0:DH], tp[:, DH:2 * DH])

            with tc.tile_pool(name="s_ps", bufs=2, space="PSUM") as s_ps, \
                 tc.tile_pool(name="o_ps", bufs=1, space="PSUM") as o_ps:
                ot_sbs = [ot_pool.tile([DA, T], FP, tag=f"ot{h}", name=f"ot{h}")
                          for h in range(HP)]
                for k in range(NK):
                    q_lo = 1024 * k
                    q_hi = 1024 * (k + 1)
                    o_tiles = [o_ps.tile([DA, 1024], FP, tag=f"o{h}", name=f"o{h}")
                               for h in range(HP)]
                    for j in range(8 * (k + 1)):
                        s0 = j * P
                        a0 = max(s0, q_lo)
                        # 512-grid chunks of the valid q range in this stripe
                        chunks = []
                        m0 = a0 // 512
                        for m in range(m0, q_hi // 512):
                            a = max(a0, m * 512)
                            e = (m + 1) * 512
                            chunks.append((a, e))
                        stl = [s_ps.tile([P, 1024], FP, tag="s", name="s")
                               for _ in range(HP)]
                        # emit head pairs adjacently: rows 0-63 (head A) and
                        # 64-127 (head B) run concurrently in the PE array
                        for (a, e) in chunks:
                            for h in range(HP):
                                hs = slice(h * DH, (h + 1) * DH)
                                nc.tensor.matmul(
                                    stl[h][:, a - q_lo:e - q_lo],
                                    kt[hs, s0:s0 + P], qt[hs, a:e],
                                    start=True, stop=True,
                                    tile_position=(h * DH, 0))
                        if q_lo <= s0:
                            for h in range(HP):
                                # diagonal block: additive causal mask
                                nc.vector.tensor_add(
                                    stl[h][:, s0 - q_lo:s0 - q_lo + P],
                                    stl[h][:, s0 - q_lo:s0 - q_lo + P],
                                    negmask[:])
                        for h in range(HP):
                            ptk = pt_pool.tile([P, 1024], BF, tag="pt")
                            nc.scalar.activation(
                                ptk[:, a0 - q_lo:1024], stl[h][:, a0 - q_lo:1024],
                                AF.Exp, scale=scale)
                            va = vaug[:, j * HP + h, :]
                            for (a, e) in chunks:
                                last_j = e // P - 1
                                nc.tensor.matmul(
                                    o_tiles[h][:, a - q_lo:e - q_lo],
                                    va, ptk[:, a - q_lo:e - q_lo],
                                    start=(j == 0), stop=(j == last_j))
                    for h in range(HP):
                        nc.vector.tensor_copy(
                            ot_sbs[h][:, q_lo:q_hi], o_tiles[h][:])
                for h in range(HP):
                    nc.sync.dma_start(out=ot_d[b, h], in_=ot_sbs[h][:])
    nc.compile()
    return nc


# --------------------------------------------------------------------------
# phase 2: per-core Wo projection + residual + rmsnorm + FFN + rmsnorm
# --------------------------------------------------------------------------

def build_phase2(NTOK, C, DFF):
    NTB = NTOK // P
    NCH = C // P
    NDF = DFF // P
    NG = DFF // 512
    halves = [(st, min(512, C - st)) for st in range(0, C, 512)]
    NH = len(halves)            # <=512-wide chunks of the channel dim

    nc = bacc.Bacc("TRN2", debug=False)
    xc_d = nc.dram_tensor("xc", [NTOK, C], FP, kind="ExternalInput").ap()
    at_d = nc.dram_tensor("attnT", [C, NTOK], BF, kind="ExternalInput").ap()
    wo_d = nc.dram_tensor("wo", [C, C], BF, kind="ExternalInput").ap()
    w1_d = nc.dram_tensor("w1", [C, DFF], BF, kind="ExternalInput").ap()
    w2_d = nc.dram_tensor("w2", [DFF, C], BF, kind="ExternalInput").ap()
    g1_d = nc.dram_tensor("g1", [C], FP, kind="ExternalInput").ap()
    g2_d = nc.dram_tensor("g2", [C], FP, kind="ExternalInput").ap()
    b1_d = nc.dram_tensor("b1", [DFF], FP, kind="ExternalInput").ap()
    b2_d = nc.dram_tensor("b2", [C], FP, kind="ExternalInput").ap()
    out_d = nc.dram_tensor("out", [NTOK, C], FP, kind="ExternalOutput").ap()

    def bcast_rows(src_ap, cols):
        # DRAM vector [cols] -> [P, cols] (same row in every partition)
        return bass.AP(tensor=src_ap.tensor, offset=src_ap.offset,
                       ap=[[0, P], [1, cols]])

    def col_ap(src_ap, start):
        # DRAM vector slice [start:start+P] -> [P, 1] (one value per partition)
        return bass.AP(tensor=src_ap.tensor, offset=src_ap.offset + start,
                       ap=[[1, P], [0, 1]])

    with tile.TileContext(nc) as tc, ExitStack() as ctx:
        const = ctx.enter_context(tc.tile_pool(name="const", bufs=1))
        work = ctx.enter_context(tc.tile_pool(name="work", bufs=2))
        stats = ctx.enter_context(tc.tile_pool(name="stats", bufs=4))
        h_pool = ctx.enter_context(tc.tile_pool(name="hp", bufs=1))
        ht_pool = ctx.enter_context(tc.tile_pool(name="htp", bufs=1))
        at_pool = ctx.enter_context(tc.tile_pool(name="atp", bufs=1))

        ident = const.tile([P, P], FP)
        make_identity(nc, ident[:])
        eps_t = const.tile([P, 1], FP)
        nc.vector.memset(eps_t[:], EPS)
        g1b = const.tile([P, C], FP)
        nc.sync.dma_start(out=g1b[:], in_=bcast_rows(g1_d, C))
        g2b = const.tile([P, C], FP)
        nc.sync.dma_start(out=g2b[:], in_=bcast_rows(g2_d, C))
        b2b = const.tile([P, C], FP)
        nc.sync.dma_start(out=b2b[:], in_=bcast_rows(b2_d, C))
        b1s = []
        for d in range(NDF):
            t = const.tile([P, 1], FP, tag=f"b1_{d}")
            nc.sync.dma_start(out=t[:], in_=col_ap(b1_d, d * P))
            b1s.append(t)

        def rmsnorm(src, gb, out_tag):
            sq = work.tile([P, C], FP, tag="sq")
            ssum = stats.tile([P, 1], FP, tag="ssum")
            nc.scalar.activation(sq[:], src[:], AF.Square, accum_out=ssum[:])
            rstd = stats.tile([P, 1], FP, tag="rstd")
            nc.scalar.activation(rstd[:], ssum[:], AF.Sqrt,
                                 scale=1.0 / C, bias=eps_t[:])
            rinv = stats.tile([P, 1], FP, tag="rinv")
            nc.vector.reciprocal(rinv[:], rstd[:])
            out = work.tile([P, C], FP, tag=out_tag)
            nc.vector.tensor_scalar_mul(out[:], src[:], rinv[:])
            nc.vector.tensor_mul(out[:], out[:], gb[:])
            return out

        # ---- stage 0: o = attnT^T @ Wo; r1 = x + o; h = rmsnorm(r1)*g1
        hs = []
        with tc.tile_pool(name="o_ps", bufs=1, space="PSUM") as o_ps, \
             tc.tile_pool(name="wop", bufs=NCH) as wop, \
             tc.tile_pool(name="atsp", bufs=NCH) as atsp, \
             tc.tile_pool(name="xcp", bufs=1) as xcp:
            atts, wots = [], []
            for c in range(NCH):
                att = atsp.tile([P, NTOK], BF, tag="at", name="at")
                nc.sync.dma_start(out=att[:], in_=at_d[c * P:(c + 1) * P, :])
                wot = wop.tile([P, C], BF, tag="wo", name="wo")
                nc.sync.dma_start(out=wot[:], in_=wo_d[c * P:(c + 1) * P, :])
                atts.append(att)
                wots.append(wot)
            xcs = []
            for tb in range(NTB):
                t = xcp.tile([P, C], FP, tag=f"xc{tb}")
                nc.sync.dma_start(out=t[:], in_=xc_d[tb * P:(tb + 1) * P, :])
                xcs.append(t)
            o_tiles = [o_ps.tile([P, 512], FP, tag=f"ops{i}", name=f"ops{i}")
                       for i in range(NTB * NH)]
            for c in range(NCH):
                att = atts[c]
                wot = wots[c]
                for tb in range(NTB):
                    for half, (hst, hw) in enumerate(halves):
                        nc.tensor.matmul(
                            o_tiles[tb * NH + half][:, :hw],
                            att[:, tb * P:(tb + 1) * P],
                            wot[:, hst:hst + hw],
                            start=(c == 0), stop=(c == NCH - 1))
            for tb in range(NTB):
                r1 = work.tile([P, C], FP, tag="r1")
                for half, (hst, hw) in enumerate(halves):
                    nc.vector.tensor_add(
                        r1[:, hst:hst + hw],
                        o_tiles[tb * NH + half][:, :hw],
                        xcs[tb][:, hst:hst + hw])
                hn = rmsnorm(r1, g1b, "hn")
                h = h_pool.tile([P, C], FP, tag=f"h{tb}")
                nc.vector.tensor_copy(h[:], hn[:])
                hs.append(h)

        # ---- stage 1: hT
        hts = [ht_pool.tile([P, NTOK], BF, tag=f"ht{c}", name=f"ht{c}")
               for c in range(NCH)]
        with tc.tile_pool(name="t_ps", bufs=4, space="PSUM") as t_ps:
            for tb in range(NTB):
                for c in range(NCH):
                    tp = t_ps.tile([P, P], FP, tag="tp")
                    nc.tensor.transpose(
                        tp[:], hs[tb][:, c * P:(c + 1) * P], ident[:])
                    nc.vector.tensor_copy(hts[c][:, tb * P:(tb + 1) * P], tp[:])

        # ---- stage 2: aT = silu(W1^T @ h^T + b1)
        ats = []
        w2p = ctx.enter_context(tc.tile_pool(name="w2p", bufs=5))
        with tc.tile_pool(name="a_ps", bufs=8, space="PSUM") as a_ps, \
             tc.tile_pool(name="w1p", bufs=5) as w1p, \
             tc.tile_pool(name="sgp", bufs=3) as sgp:
            for g in range(NG):
                aps = [a_ps.tile([P, NTOK], FP, tag="a", name="a") for _ in range(4)]
                for c in range(NCH):
                    w1t = w1p.tile([P, 512], BF, tag="w1")
                    nc.sync.dma_start(
                        out=w1t[:],
                        in_=w1_d[c * P:(c + 1) * P, g * 512:(g + 1) * 512])
                    for u in range(4):
                        nc.tensor.matmul(
                            aps[u][:], w1t[:, u * P:(u + 1) * P],
                            hts[c][:],
                            start=(c == 0), stop=(c == NCH - 1))
                for u in range(4):
                    d = 4 * g + u
                    sg = sgp.tile([P, NTOK], FP, tag="sg")
                    nc.scalar.activation(sg[:], aps[u][:], AF.Sigmoid,
                                         bias=b1s[d][:], scale=1.0)
                    at_t = at_pool.tile([P, NTOK], BF, tag=f"at{d}")
                    # silu(z) for z = a + b1: (a + b1) * sigmoid(a + b1)
                    nc.vector.scalar_tensor_tensor(
                        at_t[:], aps[u][:], b1s[d][:], sg[:],
                        op0=mybir.AluOpType.add, op1=mybir.AluOpType.mult)
                    ats.append(at_t)

        # ---- stage 3: f = aT^T @ W2; r2 = h + b2 + f; out = rmsnorm(r2)*g2
        with tc.tile_pool(name="f_ps", bufs=1, space="PSUM") as f_ps:
            fts = [f_ps.tile([P, 512], FP, tag=f"f{i}", name=f"f{i}")
                   for i in range(NTB * NH)]
            for d in range(NDF):
                w2t = w2p.tile([P, C], BF, tag="w2")
                nc.sync.dma_start(out=w2t[:], in_=w2_d[d * P:(d + 1) * P, :])
                for tb in range(NTB):
                    for half, (hst, hw) in enumerate(halves):
                        nc.tensor.matmul(
                            fts[tb * NH + half][:, :hw],
                            ats[d][:, tb * P:(tb + 1) * P],
                            w2t[:, hst:hst + hw],
                            start=(d == 0), stop=(d == NDF - 1))
            for tb in range(NTB):
                hb = work.tile([P, C], FP, tag="hb")
                nc.vector.tensor_add(hb[:], hs[tb][:], b2b[:])
                r2 = work.tile([P, C], FP, tag="r2")
                for half, (hst, hw) in enumerate(halves):
                    nc.vector.tensor_add(
                        r2[:, hst:hst + hw],
                        fts[tb * NH + half][:, :hw],
                        hb[:, hst:hst + hw])
                o = rmsnorm(r2, g2b, "outt")
                nc.sync.dma_start(out=out_d[tb * P:(tb + 1) * P, :], in_=o[:])
    nc.compile()
    return nc


# --------------------------------------------------------------------------
# host orchestration
# --------------------------------------------------------------------------

_CACHE = {}


def _phase1(B, T, C, DH):
    key = ("p1", B, T, C, DH)
    if key not in _CACHE:
        _CACHE[key] = build_phase1(B, T, C, DH)
    return _CACHE[key]


def _phase2(NTOK, C, DFF):
    key = ("p2", NTOK, C, DFF)
    if key not in _CACHE:
        _CACHE[key] = build_phase2(NTOK, C, DFF)
    return _CACHE[key]


def _run(nc, in_maps):
    import os
    trace = bool(os.environ.get("KERNEL_TRACE"))
    res = run_bass_kernel_spmd(nc, in_maps, core_ids=list(range(N_CORES)),
                               trace=trace)
    LAST_EXEC_NS.append(res.exec_time_ns)
    return res.results


def kernel(x, Wq, Wk, Wv, Wo, bo, W1, b1, W2, b2, g1, g2):
    f32 = lambda a: np.ascontiguousarray(np.asarray(a), dtype=np.float32)
    x = f32(x)
    Wq, Wk, Wv, Wo, bo = f32(Wq), f32(Wk), f32(Wv), f32(Wo), f32(bo)
    W1, b1, W2, b2, g1, g2 = f32(W1), f32(b1), f32(W2), f32(b2), f32(g1), f32(g2)

    B, T, C = x.shape
    H, _, DH = Wq.shape
    HP = H // N_CORES           # heads per core (2)
    DA = DH + 1
    LAST_EXEC_NS.clear()

    # ---- phase 1
    nc1 = _phase1(B, T, C, DH)
    xT = np.ascontiguousarray(x.transpose(0, 2, 1)).astype(BF_NP)
    in1 = []
    for i in range(N_CORES):
        pq = Wq[HP * i:HP * (i + 1)].transpose(1, 0, 2).reshape(C, HP * DH)
        pk = Wk[HP * i:HP * (i + 1)].transpose(1, 0, 2).reshape(C, HP * DH)
        pv = Wv[HP * i:HP * (i + 1)].transpose(1, 0, 2).reshape(C, HP * DH)
        in1.append({"xT": xT,
                    "wq": np.ascontiguousarray(pq).astype(BF_NP),
                    "wk": np.ascontiguousarray(pk).astype(BF_NP),
                    "wv": np.ascontiguousarray(pv).astype(BF_NP)})
    res1 = _run(nc1, in1)

    attn = np.empty((B, T, C), np.float32)
    for i in range(N_CORES):
        ot = res1[i]["ot"]                    # [B, HP, DA, T]
        o = ot[:, :, :DH, :]
        den = ot[:, :, DH, :]
        on = o / den[:, :, None, :]
        for hh in range(HP):
            hcol = (HP * i + hh) * DH
            attn[:, :, hcol:hcol + DH] = on[:, hh].transpose(0, 2, 1)

    # ---- phase 2
    NTOK = B * T // N_CORES
    nc2 = _phase2(NTOK, C, W1.shape[1])
    xf = x.reshape(B * T, C) + bo             # fold bo into the residual
    af = attn.reshape(B * T, C)
    wo_bf = Wo.astype(BF_NP)
    w1_bf = W1.astype(BF_NP)
    w2_bf = W2.astype(BF_NP)
    in2 = []
    for k in range(N_CORES):
        sl = slice(k * NTOK, (k + 1) * NTOK)
        in2.append({
            "xc": np.ascontiguousarray(xf[sl]),
            "attnT": np.ascontiguousarray(af[sl].T).astype(BF_NP),
            "wo": wo_bf, "w1": w1_bf, "w2": w2_bf,
            "g1": g1, "g2": g2, "b1": b1, "b2": b2,
        })
    res2 = _run(nc2, in2)
    out = np.concatenate([res2[k]["out"] for k in range(N_CORES)], axis=0)
    return out.reshape(B, T, C)



# revision 7
# speedup vs baseline: 1.1266x; 1.1266x over previous
"""Trainium2 Bass kernel for a dense transformer block, distributed over 8
NeuronCores.

Sharding:
  phase 1 (attention): tensor-parallel over heads — each core computes 2 of
    the 16 heads end-to-end (QKV projections + causal softmax(QK^T)V), and
    returns the unnormalized per-head output O^T together with the softmax
    denominators (obtained via a ones-column appended to V).
  phase 2 (Wo + norms + FFN): data-parallel over tokens — each core handles
    512 of the 4096 token rows with replicated weights.

The host glues the phases: transposes x, normalizes/concats heads, and
re-shards tokens.  All matmuls run as float32r (full-rate fp32 PE mode).
"""

import math
from contextlib import ExitStack

import ml_dtypes
import numpy as np

BF_NP = ml_dtypes.bfloat16

import concourse.bass as bass
import concourse.mybir as mybir
import concourse.tile as tile
from concourse import bacc
from concourse.bass_utils import run_bass_kernel_spmd
from concourse.masks import make_identity, make_upper_triangular

FP = mybir.dt.float32
FPR = mybir.dt.float32r
BF = mybir.dt.bfloat16
AF = mybir.ActivationFunctionType

N_CORES = 8
P = 128
EPS = 1e-6

# exec times (ns) of the most recent kernel() call, one entry per phase, when
# tracing was enabled via BASS_TRACE=1; None entries otherwise.
LAST_EXEC_NS = []


def _fpr(ap):
    return ap.bitcast(FPR)


# --------------------------------------------------------------------------
# phase 1: per-core attention over a pair of heads
# --------------------------------------------------------------------------

def build_phase1(B, T, C, DH):
    HP = 2                      # heads per core
    DA = DH + 1                 # head dim + ones row (softmax denominator)
    NCC = C // P                # contraction chunks
    NT = T // P                 # key/value blocks of 128
    NQ = T // 512               # query chunks of 512
    NK = T // 1024              # query tiles of 1024
    scale = float(C) ** -0.5    # NOTE: reference scales by C**-0.5, not DH

    nc = bacc.Bacc("TRN2", debug=False)
    xT_d = nc.dram_tensor("xT", [B, C, T], BF, kind="ExternalInput").ap()
    wq_d = nc.dram_tensor("wq", [C, HP * DH], BF, kind="ExternalInput").ap()
    wk_d = nc.dram_tensor("wk", [C, HP * DH], BF, kind="ExternalInput").ap()
    wv_d = nc.dram_tensor("wv", [C, HP * DH], BF, kind="ExternalInput").ap()
    ot_d = nc.dram_tensor("ot", [B, HP, DA, T], FP, kind="ExternalOutput").ap()

    with tile.TileContext(nc) as tc, ExitStack() as ctx:
        const = ctx.enter_context(tc.tile_pool(name="const", bufs=1))
        xpool = ctx.enter_context(tc.tile_pool(name="xp", bufs=1))
        wpool = ctx.enter_context(tc.tile_pool(name="wp", bufs=1))
        qk_pool = ctx.enter_context(tc.tile_pool(name="qk", bufs=2))
        vt_pool = ctx.enter_context(tc.tile_pool(name="vtp", bufs=2))
        vaug_pool = ctx.enter_context(tc.tile_pool(name="vaug", bufs=2))
        pt_pool = ctx.enter_context(tc.tile_pool(name="pt", bufs=4))
        ot_pool = ctx.enter_context(tc.tile_pool(name="otp", bufs=2))

        # additive mask for the diagonal 128x128 block of S^T [s', q']:
        # 0 where q' >= s' (causal-valid), -1e30 where q' < s'
        negmask = const.tile([P, P], FP)
        nc.gpsimd.memset(negmask[:], 0.0)
        nc.gpsimd.affine_select(
            out=negmask[:], in_=negmask[:],
            compare_op=mybir.AluOpType.is_ge, fill=-1e30,
            base=0, pattern=[[1, P]], channel_multiplier=-1)
        ident = const.tile([P, P], BF)
        make_identity(nc, ident[:])
        ones_col = const.tile([P, NT * HP, 1], FP)
        nc.vector.memset(ones_col[:], 1.0)

        # weight chunks, loaded once
        wts = {}
        for name, src in (("q", wq_d), ("k", wk_d), ("v", wv_d)):
            wts[name] = []
            for c in range(NCC):
                t = wpool.tile([P, HP * DH], BF, tag=f"w{name}{c}")
                nc.sync.dma_start(out=t[:], in_=src[c * P:(c + 1) * P, :])
                wts[name].append(t)

        for b in range(B):
            xts = []
            for c in range(NCC):
                xt = xpool.tile([P, T], BF, tag=f"x{c}")
                nc.sync.dma_start(out=xt[:], in_=xT_d[b, c * P:(c + 1) * P, :])
                xts.append(xt)

            qt = qk_pool.tile([P, T], BF, tag="qt")
            kt = qk_pool.tile([P, T], BF, tag="kt")
            vaug = vaug_pool.tile([P, NT * HP, DA], BF, tag="vaug")
            # ones column per head-block (softmax denominator row of O^T)
            nc.vector.tensor_copy(vaug[:, :, DA - 1:DA], ones_col[:])

            with tc.tile_pool(name="proj_ps", bufs=3, space="PSUM") as proj_ps, \
                 tc.tile_pool(name="vt_ps", bufs=2, space="PSUM") as vt_ps:
                for wt, dst in ((wts["q"], qt), (wts["k"], kt)):
                    for n in range(NQ):
                        ps = proj_ps.tile([P, 512], FP, tag="proj")
                        for c in range(NCC):
                            nc.tensor.matmul(
                                ps[:], wt[c][:], xts[c][:, n * 512:(n + 1) * 512],
                                start=(c == 0), stop=(c == NCC - 1))
                        nc.vector.tensor_copy(dst[:, n * 512:(n + 1) * 512], ps[:])
                # V, then transpose into [s, d] layout with ones columns
                for n in range(NQ):
                    ps = proj_ps.tile([P, 512], FP, tag="proj")
                    for c in range(NCC):
                        nc.tensor.matmul(
                            ps[:], wts["v"][c][:], xts[c][:, n * 512:(n + 1) * 512],
                            start=(c == 0), stop=(c == NCC - 1))
                    vt = vt_pool.tile([P, 512], BF, tag="vt")
                    nc.vector.tensor_copy(vt[:], ps[:])
                    for u in range(4):
                        j = 4 * n + u
                        tp = vt_ps.tile([P, P], BF, tag="vtp")
                        nc.tensor.transpose(tp[:], vt[:, u * P:(u + 1) * P], ident[:])
                        nc.vector.tensor_copy(
                            vaug[:, j * HP, 0:DH], tp[:, 0:DH])
                        nc.vector.tensor_copy(
                            vaug[:, j * HP + 1, 0:DH], tp[:, DH:2 * DH])

            with tc.tile_pool(name="s_ps", bufs=2, space="PSUM") as s_ps, \
                 tc.tile_pool(name="o_ps", bufs=1, space="PSUM") as o_ps:
                ot_sbs = [ot_pool.tile([DA, T], FP, tag=f"ot{h}", name=f"ot{h}")
                          for h in range(HP)]
                for k in range(NK):
                    q_lo = 1024 * k
                    q_hi = 1024 * (k + 1)
                    o_tiles = [o_ps.tile([DA, 1024], FP, tag=f"o{h}", name=f"o{h}")
                               for h in range(HP)]
                    for j in range(8 * (k + 1)):
                        s0 = j * P
                        a0 = max(s0, q_lo)
                        # 512-grid chunks of the valid q range in this stripe
                        chunks = []
                        m0 = a0 // 512
                        for m in range(m0, q_hi // 512):
                            a = max(a0, m * 512)
                            e = (m + 1) * 512
                            chunks.append((a, e))
                        stl = [s_ps.tile([P, 1024], FP, tag="s", name="s")
                               for _ in range(HP)]
                        # emit head pairs adjacently: rows 0-63 (head A) and
                        # 64-127 (head B) run concurrently in the PE array
                        for (a, e) in chunks:
                            for h in range(HP):
                                hs = slice(h * DH, (h + 1) * DH)
                                nc.tensor.matmul(
                                    stl[h][:, a - q_lo:e - q_lo],
                                    kt[hs, s0:s0 + P], qt[hs, a:e],
                                    start=True, stop=True,
                                    tile_position=(h * DH, 0))
                        if q_lo <= s0:
                            for h in range(HP):
                                # diagonal block: additive causal mask
                                nc.vector.tensor_add(
                                    stl[h][:, s0 - q_lo:s0 - q_lo + P],
                                    stl[h][:, s0 - q_lo:s0 - q_lo + P],
                                    negmask[:])
                        for h in range(HP):
                            ptk = pt_pool.tile([P, 1024], BF, tag="pt")
                            nc.scalar.activation(
                                ptk[:, a0 - q_lo:1024], stl[h][:, a0 - q_lo:1024],
                                AF.Exp, scale=scale)
                            va = vaug[:, j * HP + h, :]
                            for (a, e) in chunks:
                                last_j = e // P - 1
                                nc.tensor.matmul(
                                    o_tiles[h][:, a - q_lo:e - q_lo],
                                    va, ptk[:, a - q_lo:e - q_lo],
                                    start=(j == 0), stop=(j == last_j))
                    for h in range(HP):
                        nc.vector.tensor_copy(
                            ot_sbs[h][:, q_lo:q_hi], o_tiles[h][:])
                for h in range(HP):
                    nc.sync.dma_start(out=ot_d[b, h], in_=ot_sbs[h][:])
    nc.compile()
    return nc


# --------------------------------------------------------------------------
# phase 2: per-core Wo projection + residual + rmsnorm + FFN + rmsnorm
#
# All weights arrive host-packed in partition-major [128, X] layouts so each
# loads with one large contiguous DMA.  Stages are pipelined per 128-token
# block: stage0 (Wo matmuls) -> rmsnorm/transpose per block overlapped with
# the next block's matmuls; stage2 streams W1 chunks while W2 prefetches;
# stage3 runs token-block-outer with W2 resident so the final rmsnorm and
# output DMA overlap the next block's matmuls.
# --------------------------------------------------------------------------

def build_phase2(NTOK, C, DFF):
    NTB = NTOK // P             # 4 token blocks
    NCH = C // P                # 8 channel chunks
    NDF = DFF // P              # 32 ff chunks
    NG = DFF // 512             # 8 W1 column groups

    nc = bacc.Bacc("TRN2", debug=False)
    att_d = nc.dram_tensor("att", [P, NCH * NTOK], BF, kind="ExternalInput").ap()
    wo_d = nc.dram_tensor("wo", [P, NCH * C], BF, kind="ExternalInput").ap()
    xc_d = nc.dram_tensor("xc", [P, NTB * C], BF, kind="ExternalInput").ap()
    w1_d = nc.dram_tensor("w1", [P, C * DFF // P], BF, kind="ExternalInput").ap()
    w2_d = nc.dram_tensor("w2", [P, C * DFF // P], BF, kind="ExternalInput").ap()
    b1c_d = nc.dram_tensor("b1c", [P, NDF], FP, kind="ExternalInput").ap()
    g1r_d = nc.dram_tensor("g1r", [P, C], FP, kind="ExternalInput").ap()
    g2r_d = nc.dram_tensor("g2r", [P, C], FP, kind="ExternalInput").ap()
    b2r_d = nc.dram_tensor("b2r", [P, C], FP, kind="ExternalInput").ap()
    out_d = nc.dram_tensor("out", [NTOK, C], BF, kind="ExternalOutput").ap()

    with tile.TileContext(nc) as tc, ExitStack() as ctx:
        const = ctx.enter_context(tc.tile_pool(name="const", bufs=1))
        stats = ctx.enter_context(tc.tile_pool(name="stats", bufs=4))
        work = ctx.enter_context(tc.tile_pool(name="work", bufs=2))
        h_pool = ctx.enter_context(tc.tile_pool(name="hp", bufs=1))
        ht_pool = ctx.enter_context(tc.tile_pool(name="htp", bufs=1))
        at_pool = ctx.enter_context(tc.tile_pool(name="atp", bufs=1))
        s0in = tc.alloc_tile_pool(name="s0in", bufs=1)

        ident = const.tile([P, P], BF)
        make_identity(nc, ident[:])
        eps_t = const.tile([P, 1], FP)
        nc.vector.memset(eps_t[:], EPS)

        # critical-path inputs first (s0in pool is released after stage 0 so
        # the W2 resident buffer can reuse its space)
        att_t = s0in.tile([P, NCH * NTOK], BF)
        nc.sync.dma_start(out=att_t[:], in_=att_d[:, :])
        wo_t = s0in.tile([P, NCH * C], BF)
        nc.sync.dma_start(out=wo_t[:], in_=wo_d[:, :])
        xc_t = s0in.tile([P, NTB * C], BF)
        nc.sync.dma_start(out=xc_t[:], in_=xc_d[:, :])
        g1b = const.tile([P, C], FP)
        nc.sync.dma_start(out=g1b[:], in_=g1r_d[:, :])
        b1c = const.tile([P, NDF], FP)
        nc.sync.dma_start(out=b1c[:], in_=b1c_d[:, :])
        g2b = const.tile([P, C], FP)
        nc.sync.dma_start(out=g2b[:], in_=g2r_d[:, :])
        b2b = const.tile([P, C], FP)
        nc.sync.dma_start(out=b2b[:], in_=b2r_d[:, :])

        def rmsnorm_to(src, gb, out_t):
            # out = src * rsqrt(mean(src^2) + eps) * g, fused into 2 ACT + 2 DVE
            ssum = stats.tile([P, 1], FP, tag="ssum")
            sq = work.tile([P, C], FP, tag="sq")
            nc.scalar.activation(sq[:], src[:], AF.Square, accum_out=ssum[:])
            rstd = stats.tile([P, 1], FP, tag="rstd")
            nc.scalar.activation(rstd[:], ssum[:], AF.Sqrt,
                                 scale=1.0 / C, bias=eps_t[:])
            rinv = stats.tile([P, 1], FP, tag="rinv")
            nc.vector.reciprocal(rinv[:], rstd[:])
            nc.vector.scalar_tensor_tensor(
                out_t[:], src[:], rinv[:], gb[:],
                op0=mybir.AluOpType.mult, op1=mybir.AluOpType.mult)

        # ---- stage 0: o = attnT^T @ Wo; h = rmsnorm(x + bo + o) * g1; hT
        hbs = []                    # h in bf16 (residual base for r2)
        hb2s = []                   # h + b2 (fp32), precomputed for stage 3
        hts = [ht_pool.tile([P, NTOK], BF, tag=f"ht{c}", name=f"ht{c}")
               for c in range(NCH)]

        def stage0_mm(tb, o_ps):
            tiles = []
            for hst in range(0, C, 512):
                ps = o_ps.tile([P, 512], FP, tag="o", name="o")
                for c in range(NCH):
                    nc.tensor.matmul(
                        ps[:],
                        att_t[:, c * NTOK + tb * P:c * NTOK + (tb + 1) * P],
                        wo_t[:, c * C + hst:c * C + hst + 512],
                        start=(c == 0), stop=(c == NCH - 1))
                tiles.append(ps)
            return tiles

        def stage0_post(tb, tiles):
            r1 = work.tile([P, C], FP, tag="r1")
            for half, hst in enumerate(range(0, C, 512)):
                nc.vector.tensor_add(
                    r1[:, hst:hst + 512], tiles[half][:],
                    xc_t[:, tb * C + hst:tb * C + hst + 512])
            hb = h_pool.tile([P, C], BF, tag=f"h{tb}", name=f"h{tb}")
            rmsnorm_to(r1, g1b, hb)
            hbs.append(hb)
            hb2 = h_pool.tile([P, C], BF, tag=f"hb2{tb}", name=f"hb2{tb}")
            nc.vector.tensor_add(hb2[:], hb[:], b2b[:])
            hb2s.append(hb2)

        def stage0_transpose(tb, t_ps):
            for c in range(NCH):
                tp = t_ps.tile([P, P], BF, tag="tp", name="tp")
                nc.tensor.transpose(
                    tp[:], hbs[tb][:, c * P:(c + 1) * P], ident[:])
                nc.vector.tensor_copy(hts[c][:, tb * P:(tb + 1) * P], tp[:])

        with tc.tile_pool(name="o_ps", bufs=4, space="PSUM") as o_ps, \
             tc.tile_pool(name="t_ps", bufs=2, space="PSUM") as t_ps:
            pend = []
            for tb in range(NTB):
                tiles = stage0_mm(tb, o_ps)
                if pend:
                    stage0_transpose(pend[0], t_ps)
                    pend.pop()
                stage0_post(tb, tiles)
                pend.append(tb)
            for tb in pend:
                stage0_transpose(tb, t_ps)
        s0in.release()

        # ---- stage 2: aT = silu(W1^T @ hT + b1)  (W1 streamed, W2 prefetched)
        ats = []
        w2_pool = ctx.enter_context(tc.tile_pool(name="w2p", bufs=1))
        w2r = w2_pool.tile([P, C * DFF // P], BF)
        with tc.tile_pool(name="a_ps", bufs=6, space="PSUM") as a_ps, \
             tc.tile_pool(name="w1p", bufs=3) as w1p, \
             tc.tile_pool(name="sgp", bufs=3) as sgp:
            GW = NCH * 512          # per-g packed width in w1
            for g in range(NG):
                w1g = w1p.tile([P, GW], BF, tag="w1")
                nc.sync.dma_start(out=w1g[:], in_=w1_d[:, g * GW:(g + 1) * GW])
                # interleave the W2 prefetch with the W1 stream
                nc.sync.dma_start(
                    out=w2r[:, g * 4096:(g + 1) * 4096],
                    in_=w2_d[:, g * 4096:(g + 1) * 4096])
                aps = [a_ps.tile([P, NTOK], FP, tag="a", name="a")
                       for _ in range(4)]
                for c in range(NCH):
                    for u in range(4):
                        nc.tensor.matmul(
                            aps[u][:],
                            w1g[:, c * 512 + u * P:c * 512 + (u + 1) * P],
                            hts[c][:],
                            start=(c == 0), stop=(c == NCH - 1))
                for u in range(4):
                    d = 4 * g + u
                    sg = sgp.tile([P, NTOK], FP, tag="sg")
                    nc.scalar.activation(sg[:], aps[u][:], AF.Sigmoid,
                                         bias=b1c[:, d:d + 1], scale=1.0)
                    at_t = at_pool.tile([P, NTOK], BF, tag=f"at{d}")
                    nc.vector.scalar_tensor_tensor(
                        at_t[:], aps[u][:], b1c[:, d:d + 1], sg[:],
                        op0=mybir.AluOpType.add, op1=mybir.AluOpType.mult)
                    ats.append(at_t)

        # ---- stage 3: f = aT^T @ W2; out = rmsnorm(h + b2 + f) * g2
        with tc.tile_pool(name="f_ps", bufs=4, space="PSUM") as f_ps:
            for tb in range(NTB):
                tiles = []
                for hst in range(0, C, 512):
                    ps = f_ps.tile([P, 512], FP, tag="f", name="f")
                    for d in range(NDF):
                        nc.tensor.matmul(
                            ps[:],
                            ats[d][:, tb * P:(tb + 1) * P],
                            w2r[:, d * C + hst:d * C + hst + 512],
                            start=(d == 0), stop=(d == NDF - 1))
                    tiles.append(ps)
                r2 = work.tile([P, C], FP, tag="r2")
                for half, hst in enumerate(range(0, C, 512)):
                    nc.vector.tensor_add(
                        r2[:, hst:hst + 512], tiles[half][:],
                        hb2s[tb][:, hst:hst + 512])
                o_bf = work.tile([P, C], BF, tag="obf")
                rmsnorm_to(r2, g2b, o_bf)
                nc.sync.dma_start(
                    out=out_d[tb * P:(tb + 1) * P, :], in_=o_bf[:])
    nc.compile()
    return nc


def build_phase2_old(NTOK, C, DFF):
    NTB = NTOK // P
    NCH = C // P
    NDF = DFF // P
    NG = DFF // 512
    halves = [(st, min(512, C - st)) for st in range(0, C, 512)]
    NH = len(halves)            # <=512-wide chunks of the channel dim

    nc = bacc.Bacc("TRN2", debug=False)
    xc_d = nc.dram_tensor("xc", [NTOK, C], FP, kind="ExternalInput").ap()
    at_d = nc.dram_tensor("attnT", [C, NTOK], BF, kind="ExternalInput").ap()
    wo_d = nc.dram_tensor("wo", [C, C], BF, kind="ExternalInput").ap()
    w1_d = nc.dram_tensor("w1", [C, DFF], BF, kind="ExternalInput").ap()
    w2_d = nc.dram_tensor("w2", [DFF, C], BF, kind="ExternalInput").ap()
    g1_d = nc.dram_tensor("g1", [C], FP, kind="ExternalInput").ap()
    g2_d = nc.dram_tensor("g2", [C], FP, kind="ExternalInput").ap()
    b1_d = nc.dram_tensor("b1", [DFF], FP, kind="ExternalInput").ap()
    b2_d = nc.dram_tensor("b2", [C], FP, kind="ExternalInput").ap()
    out_d = nc.dram_tensor("out", [NTOK, C], FP, kind="ExternalOutput").ap()

    def bcast_rows(src_ap, cols):
        # DRAM vector [cols] -> [P, cols] (same row in every partition)
        return bass.AP(tensor=src_ap.tensor, offset=src_ap.offset,
                       ap=[[0, P], [1, cols]])

    def col_ap(src_ap, start):
        # DRAM vector slice [start:start+P] -> [P, 1] (one value per partition)
        return bass.AP(tensor=src_ap.tensor, offset=src_ap.offset + start,
                       ap=[[1, P], [0, 1]])

    with tile.TileContext(nc) as tc, ExitStack() as ctx:
        const = ctx.enter_context(tc.tile_pool(name="const", bufs=1))
        work = ctx.enter_context(tc.tile_pool(name="work", bufs=2))
        stats = ctx.enter_context(tc.tile_pool(name="stats", bufs=4))
        h_pool = ctx.enter_context(tc.tile_pool(name="hp", bufs=1))
        ht_pool = ctx.enter_context(tc.tile_pool(name="htp", bufs=1))
        at_pool = ctx.enter_context(tc.tile_pool(name="atp", bufs=1))

        ident = const.tile([P, P], FP)
        make_identity(nc, ident[:])
        eps_t = const.tile([P, 1], FP)
        nc.vector.memset(eps_t[:], EPS)
        g1b = const.tile([P, C], FP)
        nc.sync.dma_start(out=g1b[:], in_=bcast_rows(g1_d, C))
        g2b = const.tile([P, C], FP)
        nc.sync.dma_start(out=g2b[:], in_=bcast_rows(g2_d, C))
        b2b = const.tile([P, C], FP)
        nc.sync.dma_start(out=b2b[:], in_=bcast_rows(b2_d, C))
        b1s = []
        for d in range(NDF):
            t = const.tile([P, 1], FP, tag=f"b1_{d}")
            nc.sync.dma_start(out=t[:], in_=col_ap(b1_d, d * P))
            b1s.append(t)

        def rmsnorm(src, gb, out_tag):
            sq = work.tile([P, C], FP, tag="sq")
            ssum = stats.tile([P, 1], FP, tag="ssum")
            nc.scalar.activation(sq[:], src[:], AF.Square, accum_out=ssum[:])
            rstd = stats.tile([P, 1], FP, tag="rstd")
            nc.scalar.activation(rstd[:], ssum[:], AF.Sqrt,
                                 scale=1.0 / C, bias=eps_t[:])
            rinv = stats.tile([P, 1], FP, tag="rinv")
            nc.vector.reciprocal(rinv[:], rstd[:])
            out = work.tile([P, C], FP, tag=out_tag)
            nc.vector.tensor_scalar_mul(out[:], src[:], rinv[:])
            nc.vector.tensor_mul(out[:], out[:], gb[:])
            return out

        # ---- stage 0: o = attnT^T @ Wo; r1 = x + o; h = rmsnorm(r1)*g1
        hs = []
        with tc.tile_pool(name="o_ps", bufs=1, space="PSUM") as o_ps, \
             tc.tile_pool(name="wop", bufs=NCH) as wop, \
             tc.tile_pool(name="atsp", bufs=NCH) as atsp, \
             tc.tile_pool(name="xcp", bufs=1) as xcp:
            atts, wots = [], []
            for c in range(NCH):
                att = atsp.tile([P, NTOK], BF, tag="at", name="at")
                nc.sync.dma_start(out=att[:], in_=at_d[c * P:(c + 1) * P, :])
                wot = wop.tile([P, C], BF, tag="wo", name="wo")
                nc.sync.dma_start(out=wot[:], in_=wo_d[c * P:(c + 1) * P, :])
                atts.append(att)
                wots.append(wot)
            xcs = []
            for tb in range(NTB):
                t = xcp.tile([P, C], FP, tag=f"xc{tb}")
                nc.sync.dma_start(out=t[:], in_=xc_d[tb * P:(tb + 1) * P, :])
                xcs.append(t)
            o_tiles = [o_ps.tile([P, 512], FP, tag=f"ops{i}", name=f"ops{i}")
                       for i in range(NTB * NH)]
            for c in range(NCH):
                att = atts[c]
                wot = wots[c]
                for tb in range(NTB):
                    for half, (hst, hw) in enumerate(halves):
                        nc.tensor.matmul(
                            o_tiles[tb * NH + half][:, :hw],
                            att[:, tb * P:(tb + 1) * P],
                            wot[:, hst:hst + hw],
                            start=(c == 0), stop=(c == NCH - 1))
            for tb in range(NTB):
                r1 = work.tile([P, C], FP, tag="r1")
                for half, (hst, hw) in enumerate(halves):
                    nc.vector.tensor_add(
                        r1[:, hst:hst + hw],
                        o_tiles[tb * NH + half][:, :hw],
                        xcs[tb][:, hst:hst + hw])
                hn = rmsnorm(r1, g1b, "hn")
                h = h_pool.tile([P, C], FP, tag=f"h{tb}")
                nc.vector.tensor_copy(h[:], hn[:])
                hs.append(h)

        # ---- stage 1: hT
        hts = [ht_pool.tile([P, NTOK], BF, tag=f"ht{c}", name=f"ht{c}")
               for c in range(NCH)]
        with tc.tile_pool(name="t_ps", bufs=4, space="PSUM") as t_ps:
            for tb in range(NTB):
                for c in range(NCH):
                    tp = t_ps.tile([P, P], FP, tag="tp")
                    nc.tensor.transpose(
                        tp[:], hs[tb][:, c * P:(c + 1) * P], ident[:])
                    nc.vector.tensor_copy(hts[c][:, tb * P:(tb + 1) * P], tp[:])

        # ---- stage 2: aT = silu(W1^T @ h^T + b1)
        ats = []
        w2p = ctx.enter_context(tc.tile_pool(name="w2p", bufs=5))
        with tc.tile_pool(name="a_ps", bufs=8, space="PSUM") as a_ps, \
             tc.tile_pool(name="w1p", bufs=5) as w1p, \
             tc.tile_pool(name="sgp", bufs=3) as sgp:
            for g in range(NG):
                aps = [a_ps.tile([P, NTOK], FP, tag="a", name="a") for _ in range(4)]
                for c in range(NCH):
                    w1t = w1p.tile([P, 512], BF, tag="w1")
                    nc.sync.dma_start(
                        out=w1t[:],
                        in_=w1_d[c * P:(c + 1) * P, g * 512:(g + 1) * 512])
                    for u in range(4):
                        nc.tensor.matmul(
                            aps[u][:], w1t[:, u * P:(u + 1) * P],
                            hts[c][:],
                            start=(c == 0), stop=(c == NCH - 1))
                for u in range(4):
                    d = 4 * g + u
                    sg = sgp.tile([P, NTOK], FP, tag="sg")
                    nc.scalar.activation(sg[:], aps[u][:], AF.Sigmoid,
                                         bias=b1s[d][:], scale=1.0)
                    at_t = at_pool.tile([P, NTOK], BF, tag=f"at{d}")
                    # silu(z) for z = a + b1: (a + b1) * sigmoid(a + b1)
                    nc.vector.scalar_tensor_tensor(
                        at_t[:], aps[u][:], b1s[d][:], sg[:],
                        op0=mybir.AluOpType.add, op1=mybir.AluOpType.mult)
                    ats.append(at_t)

        # ---- stage 3: f = aT^T @ W2; r2 = h + b2 + f; out = rmsnorm(r2)*g2
        with tc.tile_pool(name="f_ps", bufs=1, space="PSUM") as f_ps:
            fts = [f_ps.tile([P, 512], FP, tag=f"f{i}", name=f"f{i}")
                   for i in range(NTB * NH)]
            for d in range(NDF):
                w2t = w2p.tile([P, C], BF, tag="w2")
                nc.sync.dma_start(out=w2t[:], in_=w2_d[d * P:(d + 1) * P, :])
                for tb in range(NTB):
                    for half, (hst, hw) in enumerate(halves):
                        nc.tensor.matmul(
                            fts[tb * NH + half][:, :hw],
                            ats[d][:, tb * P:(tb + 1) * P],
                            w2t[:, hst:hst + hw],
                            start=(d == 0), stop=(d == NDF - 1))
            for tb in range(NTB):
                hb = work.tile([P, C], FP, tag="hb")
                nc.vector.tensor_add(hb[:], hs[tb][:], b2b[:])
                r2 = work.tile([P, C], FP, tag="r2")
                for half, (hst, hw) in enumerate(halves):
                    nc.vector.tensor_add(
                        r2[:, hst:hst + hw],
                        fts[tb * NH + half][:, :hw],
                        hb[:, hst:hst + hw])
                o = rmsnorm(r2, g2b, "outt")
                nc.sync.dma_start(out=out_d[tb * P:(tb + 1) * P, :], in_=o[:])
    nc.compile()
    return nc


# --------------------------------------------------------------------------
# host orchestration
# --------------------------------------------------------------------------

_CACHE = {}


def _phase1(B, T, C, DH):
    key = ("p1", B, T, C, DH)
    if key not in _CACHE:
        _CACHE[key] = build_phase1(B, T, C, DH)
    return _CACHE[key]


def _phase2(NTOK, C, DFF):
    key = ("p2", NTOK, C, DFF)
    if key not in _CACHE:
        _CACHE[key] = build_phase2(NTOK, C, DFF)
    return _CACHE[key]


def _run(nc, in_maps):
    import os
    trace = bool(os.environ.get("KERNEL_TRACE"))
    res = run_bass_kernel_spmd(nc, in_maps, core_ids=list(range(N_CORES)),
                               trace=trace)
    LAST_EXEC_NS.append(res.exec_time_ns)
    return res.results


def kernel(x, Wq, Wk, Wv, Wo, bo, W1, b1, W2, b2, g1, g2):
    f32 = lambda a: np.ascontiguousarray(np.asarray(a), dtype=np.float32)
    x = f32(x)
    Wq, Wk, Wv, Wo, bo = f32(Wq), f32(Wk), f32(Wv), f32(Wo), f32(bo)
    W1, b1, W2, b2, g1, g2 = f32(W1), f32(b1), f32(W2), f32(b2), f32(g1), f32(g2)

    B, T, C = x.shape
    H, _, DH = Wq.shape
    HP = H // N_CORES           # heads per core (2)
    DA = DH + 1
    LAST_EXEC_NS.clear()

    # ---- phase 1
    nc1 = _phase1(B, T, C, DH)
    xT = np.ascontiguousarray(x.transpose(0, 2, 1)).astype(BF_NP)
    in1 = []
    for i in range(N_CORES):
        pq = Wq[HP * i:HP * (i + 1)].transpose(1, 0, 2).reshape(C, HP * DH)
        pk = Wk[HP * i:HP * (i + 1)].transpose(1, 0, 2).reshape(C, HP * DH)
        pv = Wv[HP * i:HP * (i + 1)].transpose(1, 0, 2).reshape(C, HP * DH)
        in1.append({"xT": xT,
                    "wq": np.ascontiguousarray(pq).astype(BF_NP),
                    "wk": np.ascontiguousarray(pk).astype(BF_NP),
                    "wv": np.ascontiguousarray(pv).astype(BF_NP)})
    res1 = _run(nc1, in1)

    attn = np.empty((B, T, C), np.float32)
    for i in range(N_CORES):
        ot = res1[i]["ot"]                    # [B, HP, DA, T]
        o = ot[:, :, :DH, :]
        den = ot[:, :, DH, :]
        on = o / den[:, :, None, :]
        for hh in range(HP):
            hcol = (HP * i + hh) * DH
            attn[:, :, hcol:hcol + DH] = on[:, hh].transpose(0, 2, 1)

    # ---- phase 2
    NTOK = B * T // N_CORES
    DFF = W1.shape[1]
    NTB, NCH, NDF = NTOK // 128, C // 128, DFF // 128
    nc2 = _phase2(NTOK, C, DFF)
    xf = x.reshape(B * T, C) + bo             # fold bo into the residual
    af = attn.reshape(B * T, C)
    # partition-major packs: one big contiguous DMA per tensor on device
    wo_p = np.ascontiguousarray(
        Wo.reshape(NCH, 128, C).transpose(1, 0, 2).reshape(128, NCH * C)
    ).astype(BF_NP)
    w1_p = np.ascontiguousarray(
        W1.reshape(NCH, 128, DFF // 512, 512).transpose(1, 2, 0, 3)
        .reshape(128, C * DFF // 128)).astype(BF_NP)
    w2_p = np.ascontiguousarray(
        W2.reshape(NDF, 128, C).transpose(1, 0, 2).reshape(128, DFF * C // 128)
    ).astype(BF_NP)
    b1c = np.ascontiguousarray(b1.reshape(NDF, 128).T)
    g1r = np.ascontiguousarray(np.broadcast_to(g1, (128, C)))
    g2r = np.ascontiguousarray(np.broadcast_to(g2, (128, C)))
    b2r = np.ascontiguousarray(np.broadcast_to(b2, (128, C)))
    in2 = []
    for k in range(N_CORES):
        sl = slice(k * NTOK, (k + 1) * NTOK)
        att_p = np.ascontiguousarray(
            af[sl].T.reshape(NCH, 128, NTOK).transpose(1, 0, 2)
            .reshape(128, NCH * NTOK)).astype(BF_NP)
        xc_p = np.ascontiguousarray(
            xf[sl].reshape(NTB, 128, C).transpose(1, 0, 2)
            .reshape(128, NTB * C)).astype(BF_NP)
        in2.append({
            "att": att_p, "xc": xc_p, "wo": wo_p, "w1": w1_p, "w2": w2_p,
            "b1c": b1c, "g1r": g1r, "g2r": g2r, "b2r": b2r,
        })
    res2 = _run(nc2, in2)
    out = np.concatenate(
        [res2[k]["out"].astype(np.float32) for k in range(N_CORES)], axis=0)
    return out.reshape(B, T, C)



# revision 10
# speedup vs baseline: 1.4175x; 1.2582x over previous
"""Trainium2 Bass kernel for a dense transformer block, distributed over 8
NeuronCores.

Sharding:
  phase 1 (attention): tensor-parallel over heads — each core computes 2 of
    the 16 heads end-to-end (QKV projections + causal softmax(QK^T)V), and
    returns the unnormalized per-head output O^T together with the softmax
    denominators (obtained via a ones-column appended to V).
  phase 2 (Wo + norms + FFN): data-parallel over tokens — each core handles
    512 of the 4096 token rows with replicated weights.

The host glues the phases: transposes x, normalizes/concats heads, and
re-shards tokens.  All matmuls run as float32r (full-rate fp32 PE mode).
"""

import math
from contextlib import ExitStack

import ml_dtypes
import numpy as np

BF_NP = ml_dtypes.bfloat16

import concourse.bass as bass
import concourse.mybir as mybir
import concourse.tile as tile
from concourse import bacc
from concourse.bass_utils import run_bass_kernel_spmd
from concourse.masks import make_identity, make_upper_triangular

FP = mybir.dt.float32
FPR = mybir.dt.float32r
BF = mybir.dt.bfloat16
AF = mybir.ActivationFunctionType

N_CORES = 8
P = 128
EPS = 1e-6

# exec times (ns) of the most recent kernel() call, one entry per phase, when
# tracing was enabled via BASS_TRACE=1; None entries otherwise.
LAST_EXEC_NS = []


def _fpr(ap):
    return ap.bitcast(FPR)


# --------------------------------------------------------------------------
# phase 1: per-core attention over a pair of heads
#
# Software-pipelined: per key-stripe the PE emits S(j) while ACT runs exp(j)
# and the PE retires AV(j-1), so the exp latency never blocks the PE queue.
# Batch 1's QKV projection matmuls are interleaved as PE "filler" under
# batch 0's ACT-bound softmax stretches.  Per-stripe exp covers both heads
# in a single ACT call (one 352-cycle startup instead of two).
# --------------------------------------------------------------------------

def build_phase1(B, T, C, DH):
    HP = 2                      # heads per core
    DA = DH + 1                 # head dim + ones row (softmax denominator)
    NCC = C // P                # contraction chunks (8)
    NQ = T // 512               # q-tiles / projection column chunks (4)
    scale = float(C) ** -0.5

    nc = bacc.Bacc("TRN2", debug=False)
    xT_d = nc.dram_tensor("xT", [B, C, T], BF, kind="ExternalInput").ap()
    wq_d = nc.dram_tensor("wq", [P, NCC * P], BF, kind="ExternalInput").ap()
    wk_d = nc.dram_tensor("wk", [P, NCC * P], BF, kind="ExternalInput").ap()
    wv_d = nc.dram_tensor("wv", [P, NCC * P], BF, kind="ExternalInput").ap()
    ot_d = nc.dram_tensor("ot", [B, HP, DA, T], BF, kind="ExternalOutput").ap()

    with tile.TileContext(nc) as tc, ExitStack() as ctx:
        const = ctx.enter_context(tc.tile_pool(name="const", bufs=1))
        wpool = ctx.enter_context(tc.tile_pool(name="wp", bufs=1))
        xpool = ctx.enter_context(tc.tile_pool(name="xp", bufs=1))
        qk_pool = ctx.enter_context(tc.tile_pool(name="qk", bufs=1))
        vt_pool = ctx.enter_context(tc.tile_pool(name="vtp", bufs=2))
        vaug_pool = ctx.enter_context(tc.tile_pool(name="vaug", bufs=1))
        pt_pool = ctx.enter_context(tc.tile_pool(name="ptp", bufs=3))
        ot_pool = ctx.enter_context(tc.tile_pool(name="otp", bufs=1))
        proj_ps = ctx.enter_context(
            tc.tile_pool(name="proj_ps", bufs=1, space="PSUM"))
        vt_ps = ctx.enter_context(
            tc.tile_pool(name="vt_ps", bufs=1, space="PSUM"))
        s_ps = ctx.enter_context(
            tc.tile_pool(name="s_ps", bufs=2, space="PSUM"))
        o_ps = ctx.enter_context(
            tc.tile_pool(name="o_ps", bufs=1, space="PSUM"))

        # warm the ACT exp table while DMAs run
        warm = const.tile([P, 1], FP)
        nc.vector.memset(warm[:], 0.0)
        nc.scalar.activation(warm[:], warm[:], AF.Exp)

        negmask = const.tile([P, P], FP)
        nc.gpsimd.memset(negmask[:], 0.0)
        nc.gpsimd.affine_select(
            out=negmask[:], in_=negmask[:],
            compare_op=mybir.AluOpType.is_ge, fill=-1e30,
            base=0, pattern=[[1, P]], channel_multiplier=-1)
        ident = const.tile([P, P], BF)
        make_identity(nc, ident[:])

        wts = {}
        for name, src in (("q", wq_d), ("k", wk_d), ("v", wv_d)):
            t = wpool.tile([P, NCC * P], BF, tag=f"w{name}", name=f"w{name}")
            nc.sync.dma_start(out=t[:], in_=src[:, :])
            wts[name] = t

        xts = {b: [xpool.tile([P, T], BF, tag=f"x{b}{c}", name=f"x{b}{c}")
                   for c in range(NCC)] for b in range(B)}
        for half in range(2):
            for b in range(B):
                for c in range(NCC):
                    nc.sync.dma_start(
                        out=xts[b][c][:, half * 1024:(half + 1) * 1024],
                        in_=xT_d[b, c * P:(c + 1) * P,
                                 half * 1024:(half + 1) * 1024])

        qts = {b: qk_pool.tile([P, T], BF, tag=f"q{b}", name=f"q{b}")
               for b in range(B)}
        kts = {b: qk_pool.tile([P, T], BF, tag=f"k{b}", name=f"k{b}")
               for b in range(B)}
        vaugs = {b: vaug_pool.tile([P, (T // P) * HP, DA], BF,
                                   tag=f"va{b}", name=f"va{b}")
                 for b in range(B)}
        for b in range(B):
            nc.vector.memset(vaugs[b][:, :, DA - 1:DA], 1.0)
        ot_sbs = {(b, h): ot_pool.tile([DA, T], BF, tag=f"ot{b}{h}",
                                       name=f"ot{b}{h}")
                  for b in range(B) for h in range(HP)}

        def qkv_gen(b, n):
            """Emit QKV projections for 512-token chunk n of batch b, in
            small quanta so it can interleave with attention emission."""
            cs = slice(n * 512, (n + 1) * 512)
            for wt, dst in ((wts["q"], qts[b]), (wts["k"], kts[b])):
                ps = proj_ps.tile([P, 512], FP, tag="proj", name="proj")
                for c in range(NCC):
                    nc.tensor.matmul(
                        ps[:], wt[:, c * P:(c + 1) * P], xts[b][c][:, cs],
                        start=(c == 0), stop=(c == NCC - 1))
                    if c % 2 == 1:
                        yield
                nc.vector.tensor_copy(dst[:, cs], ps[:])
                yield
            ps = proj_ps.tile([P, 512], FP, tag="proj", name="proj")
            for c in range(NCC):
                nc.tensor.matmul(
                    ps[:], wts["v"][:, c * P:(c + 1) * P], xts[b][c][:, cs],
                    start=(c == 0), stop=(c == NCC - 1))
                if c % 2 == 1:
                    yield
            vt = vt_pool.tile([P, 512], BF, tag="vt")
            nc.vector.tensor_copy(vt[:], ps[:])
            yield
            for u in range(4):
                kb = 4 * n + u
                tp = vt_ps.tile([P, P], BF, tag="vtp", name="vtp")
                nc.tensor.transpose(tp[:], vt[:, u * P:(u + 1) * P], ident[:])
                nc.vector.tensor_copy(vaugs[b][:, kb * HP, 0:DH], tp[:, 0:DH])
                nc.vector.tensor_copy(
                    vaugs[b][:, kb * HP + 1, 0:DH], tp[:, DH:2 * DH])
                yield

        def drain(gens, k=None):
            done = 0
            while gens and (k is None or done < k):
                try:
                    next(gens[0])
                    done += 1
                except StopIteration:
                    gens.pop(0)

        # pending AV state for cross-stripe software pipelining
        pend_av = []

        def flush_av():
            for emit in pend_av:
                emit()
            pend_av.clear()

        def attn_tile(b, t, fillers, k_per_stripe, after_first=None):
            nstr = 4 * t + 4
            o_t = o_ps.tile([DA, HP, 512], FP, tag="o", name="o")
            for j in range(nstr):
                s0 = P * j
                off = max(0, s0 - 512 * t)
                sp = s_ps.tile([P, HP, 512], FP, tag="s", name="s")
                for h in range(HP):
                    nc.tensor.matmul(
                        sp[:, h, off:512],
                        kts[b][h * DH:(h + 1) * DH, s0:s0 + P],
                        qts[b][h * DH:(h + 1) * DH,
                               512 * t + off:512 * (t + 1)],
                        start=True, stop=True, tile_position=(h * DH, 0))
                if s0 >= 512 * t:
                    for h in range(HP):
                        nc.vector.tensor_add(
                            sp[:, h, off:off + P], sp[:, h, off:off + P],
                            negmask[:])
                pt = pt_pool.tile([P, HP, 512], BF, tag="pt")
                nc.scalar.activation(
                    pt[:, :, off:512], sp[:, :, off:512], AF.Exp, scale=scale)
                flush_av()
                if j == 0 and after_first is not None:
                    after_first()

                def av(b=b, j=j, off=off, pt=pt, o_t=o_t,
                       start=(j == 0), stop=(j == nstr - 1)):
                    for h in range(HP):
                        nc.tensor.matmul(
                            o_t[:, h, off:512],
                            vaugs[b][:, j * HP + h, :],
                            pt[:, h, off:512],
                            start=start, stop=stop)
                pend_av.append(av)
                drain(fillers, k_per_stripe)
            return o_t

        def finish_tile(b, t, o_t):
            for h in range(HP):
                nc.vector.tensor_copy(
                    ot_sbs[(b, h)][:, 512 * t:512 * (t + 1)], o_t[:, h, :])

        # upfront QKV for chunk 0 of both batches
        drain([qkv_gen(0, 0)])
        fillers = [qkv_gen(1, 0)]
        o00 = attn_tile(0, 0, fillers, 6)
        drain(fillers)                      # b1 chunk-0 QKV must be done
        fillers = [qkv_gen(b, n) for n in range(1, NQ) for b in range(B)]
        nfill = len(fillers)

        prev = (0, 0, o00)
        order = [(1, 0)] + [(b, t) for t in range(1, NQ) for b in range(B)]
        for b, t in order:
            if t >= 1:
                # qkv_gen(b, n=t) sits at index 2*(t-1)+b of the filler list;
                # it must be fully emitted before attn(b, t)
                keep = nfill - (2 * (t - 1) + b) - 1
                while len(fillers) > keep:
                    drain(fillers, 4)
            pv = prev

            def cb(pv=pv):
                finish_tile(*pv)
            o_t = attn_tile(b, t, fillers, 6 if t == 0 else 2, after_first=cb)
            prev = (b, t, o_t)
        flush_av()
        finish_tile(*prev)
        drain(fillers)

        for b in range(B):
            for h in range(HP):
                nc.sync.dma_start(out=ot_d[b, h], in_=ot_sbs[(b, h)][:])
    nc.compile()
    return nc


def build_phase1_old(B, T, C, DH):
    HP = 2                      # heads per core
    DA = DH + 1                 # head dim + ones row (softmax denominator)
    NCC = C // P                # contraction chunks
    NT = T // P                 # key/value blocks of 128
    NQ = T // 512               # query chunks of 512
    NK = T // 1024              # query tiles of 1024
    scale = float(C) ** -0.5    # NOTE: reference scales by C**-0.5, not DH

    nc = bacc.Bacc("TRN2", debug=False)
    xT_d = nc.dram_tensor("xT", [B, C, T], BF, kind="ExternalInput").ap()
    wq_d = nc.dram_tensor("wq", [C, HP * DH], BF, kind="ExternalInput").ap()
    wk_d = nc.dram_tensor("wk", [C, HP * DH], BF, kind="ExternalInput").ap()
    wv_d = nc.dram_tensor("wv", [C, HP * DH], BF, kind="ExternalInput").ap()
    ot_d = nc.dram_tensor("ot", [B, HP, DA, T], FP, kind="ExternalOutput").ap()

    with tile.TileContext(nc) as tc, ExitStack() as ctx:
        const = ctx.enter_context(tc.tile_pool(name="const", bufs=1))
        xpool = ctx.enter_context(tc.tile_pool(name="xp", bufs=1))
        wpool = ctx.enter_context(tc.tile_pool(name="wp", bufs=1))
        qk_pool = ctx.enter_context(tc.tile_pool(name="qk", bufs=2))
        vt_pool = ctx.enter_context(tc.tile_pool(name="vtp", bufs=2))
        vaug_pool = ctx.enter_context(tc.tile_pool(name="vaug", bufs=2))
        pt_pool = ctx.enter_context(tc.tile_pool(name="pt", bufs=4))
        ot_pool = ctx.enter_context(tc.tile_pool(name="otp", bufs=2))

        # additive mask for the diagonal 128x128 block of S^T [s', q']:
        # 0 where q' >= s' (causal-valid), -1e30 where q' < s'
        negmask = const.tile([P, P], FP)
        nc.gpsimd.memset(negmask[:], 0.0)
        nc.gpsimd.affine_select(
            out=negmask[:], in_=negmask[:],
            compare_op=mybir.AluOpType.is_ge, fill=-1e30,
            base=0, pattern=[[1, P]], channel_multiplier=-1)
        ident = const.tile([P, P], BF)
        make_identity(nc, ident[:])
        ones_col = const.tile([P, NT * HP, 1], FP)
        nc.vector.memset(ones_col[:], 1.0)

        # weight chunks, loaded once
        wts = {}
        for name, src in (("q", wq_d), ("k", wk_d), ("v", wv_d)):
            wts[name] = []
            for c in range(NCC):
                t = wpool.tile([P, HP * DH], BF, tag=f"w{name}{c}")
                nc.sync.dma_start(out=t[:], in_=src[c * P:(c + 1) * P, :])
                wts[name].append(t)

        for b in range(B):
            xts = []
            for c in range(NCC):
                xt = xpool.tile([P, T], BF, tag=f"x{c}")
                nc.sync.dma_start(out=xt[:], in_=xT_d[b, c * P:(c + 1) * P, :])
                xts.append(xt)

            qt = qk_pool.tile([P, T], BF, tag="qt")
            kt = qk_pool.tile([P, T], BF, tag="kt")
            vaug = vaug_pool.tile([P, NT * HP, DA], BF, tag="vaug")
            # ones column per head-block (softmax denominator row of O^T)
            nc.vector.tensor_copy(vaug[:, :, DA - 1:DA], ones_col[:])

            with tc.tile_pool(name="proj_ps", bufs=3, space="PSUM") as proj_ps, \
                 tc.tile_pool(name="vt_ps", bufs=2, space="PSUM") as vt_ps:
                for wt, dst in ((wts["q"], qt), (wts["k"], kt)):
                    for n in range(NQ):
                        ps = proj_ps.tile([P, 512], FP, tag="proj")
                        for c in range(NCC):
                            nc.tensor.matmul(
                                ps[:], wt[c][:], xts[c][:, n * 512:(n + 1) * 512],
                                start=(c == 0), stop=(c == NCC - 1))
                        nc.vector.tensor_copy(dst[:, n * 512:(n + 1) * 512], ps[:])
                # V, then transpose into [s, d] layout with ones columns
                for n in range(NQ):
                    ps = proj_ps.tile([P, 512], FP, tag="proj")
                    for c in range(NCC):
                        nc.tensor.matmul(
                            ps[:], wts["v"][c][:], xts[c][:, n * 512:(n + 1) * 512],
                            start=(c == 0), stop=(c == NCC - 1))
                    vt = vt_pool.tile([P, 512], BF, tag="vt")
                    nc.vector.tensor_copy(vt[:], ps[:])
                    for u in range(4):
                        j = 4 * n + u
                        tp = vt_ps.tile([P, P], BF, tag="vtp")
                        nc.tensor.transpose(tp[:], vt[:, u * P:(u + 1) * P], ident[:])
                        nc.vector.tensor_copy(
                            vaug[:, j * HP, 0:DH], tp[:, 0:DH])
                        nc.vector.tensor_copy(
                            vaug[:, j * HP + 1, 0:DH], tp[:, DH:2 * DH])

            with tc.tile_pool(name="s_ps", bufs=2, space="PSUM") as s_ps, \
                 tc.tile_pool(name="o_ps", bufs=1, space="PSUM") as o_ps:
                ot_sbs = [ot_pool.tile([DA, T], FP, tag=f"ot{h}", name=f"ot{h}")
                          for h in range(HP)]
                for k in range(NK):
                    q_lo = 1024 * k
                    q_hi = 1024 * (k + 1)
                    o_tiles = [o_ps.tile([DA, 1024], FP, tag=f"o{h}", name=f"o{h}")
                               for h in range(HP)]
                    for j in range(8 * (k + 1)):
                        s0 = j * P
                        a0 = max(s0, q_lo)
                        # 512-grid chunks of the valid q range in this stripe
                        chunks = []
                        m0 = a0 // 512
                        for m in range(m0, q_hi // 512):
                            a = max(a0, m * 512)
                            e = (m + 1) * 512
                            chunks.append((a, e))
                        stl = [s_ps.tile([P, 1024], FP, tag="s", name="s")
                               for _ in range(HP)]
                        # emit head pairs adjacently: rows 0-63 (head A) and
                        # 64-127 (head B) run concurrently in the PE array
                        for (a, e) in chunks:
                            for h in range(HP):
                                hs = slice(h * DH, (h + 1) * DH)
                                nc.tensor.matmul(
                                    stl[h][:, a - q_lo:e - q_lo],
                                    kt[hs, s0:s0 + P], qt[hs, a:e],
                                    start=True, stop=True,
                                    tile_position=(h * DH, 0))
                        if q_lo <= s0:
                            for h in range(HP):
                                # diagonal block: additive causal mask
                                nc.vector.tensor_add(
                                    stl[h][:, s0 - q_lo:s0 - q_lo + P],
                                    stl[h][:, s0 - q_lo:s0 - q_lo + P],
                                    negmask[:])
                        for h in range(HP):
                            ptk = pt_pool.tile([P, 1024], BF, tag="pt")
                            nc.scalar.activation(
                                ptk[:, a0 - q_lo:1024], stl[h][:, a0 - q_lo:1024],
                                AF.Exp, scale=scale)
                            va = vaug[:, j * HP + h, :]
                            for (a, e) in chunks:
                                last_j = e // P - 1
                                nc.tensor.matmul(
                                    o_tiles[h][:, a - q_lo:e - q_lo],
                                    va, ptk[:, a - q_lo:e - q_lo],
                                    start=(j == 0), stop=(j == last_j))
                    for h in range(HP):
                        nc.vector.tensor_copy(
                            ot_sbs[h][:, q_lo:q_hi], o_tiles[h][:])
                for h in range(HP):
                    nc.sync.dma_start(out=ot_d[b, h], in_=ot_sbs[h][:])
    nc.compile()
    return nc


# --------------------------------------------------------------------------
# phase 2: per-core Wo projection + residual + rmsnorm + FFN + rmsnorm
#
# All weights arrive host-packed in partition-major [128, X] layouts so each
# loads with one large contiguous DMA.  Stages are pipelined per 128-token
# block: stage0 (Wo matmuls) -> rmsnorm/transpose per block overlapped with
# the next block's matmuls; stage2 streams W1 chunks while W2 prefetches;
# stage3 runs token-block-outer with W2 resident so the final rmsnorm and
# output DMA overlap the next block's matmuls.
# --------------------------------------------------------------------------

def build_phase2(NTOK, C, DFF):
    NTB = NTOK // P             # 4 token blocks
    NCH = C // P                # 8 channel chunks
    NDF = DFF // P              # 32 ff chunks
    NG = DFF // 512             # 8 W1 column groups

    nc = bacc.Bacc("TRN2", debug=False)
    att_d = nc.dram_tensor("att", [P, NCH * NTOK], BF, kind="ExternalInput").ap()
    wo_d = nc.dram_tensor("wo", [P, NCH * C], BF, kind="ExternalInput").ap()
    xc_d = nc.dram_tensor("xc", [P, NTB * C], BF, kind="ExternalInput").ap()
    w1_d = nc.dram_tensor("w1", [P, C * DFF // P], BF, kind="ExternalInput").ap()
    w2_d = nc.dram_tensor("w2", [P, C * DFF // P], BF, kind="ExternalInput").ap()
    b1c_d = nc.dram_tensor("b1c", [P, NDF], FP, kind="ExternalInput").ap()
    g1r_d = nc.dram_tensor("g1r", [P, C], FP, kind="ExternalInput").ap()
    g2r_d = nc.dram_tensor("g2r", [P, C], FP, kind="ExternalInput").ap()
    b2r_d = nc.dram_tensor("b2r", [P, C], FP, kind="ExternalInput").ap()
    out_d = nc.dram_tensor("out", [NTOK, C], BF, kind="ExternalOutput").ap()

    with tile.TileContext(nc) as tc, ExitStack() as ctx:
        const = ctx.enter_context(tc.tile_pool(name="const", bufs=1))
        stats = ctx.enter_context(tc.tile_pool(name="stats", bufs=4))
        work = ctx.enter_context(tc.tile_pool(name="work", bufs=2))
        h_pool = ctx.enter_context(tc.tile_pool(name="hp", bufs=1))
        ht_pool = ctx.enter_context(tc.tile_pool(name="htp", bufs=1))
        at_pool = ctx.enter_context(tc.tile_pool(name="atp", bufs=1))
        s0in = tc.alloc_tile_pool(name="s0in", bufs=1)

        ident = const.tile([P, P], BF)
        make_identity(nc, ident[:])
        eps_t = const.tile([P, 1], FP)
        nc.vector.memset(eps_t[:], EPS)

        # critical-path inputs first (s0in pool is released after stage 0 so
        # the W2 resident buffer can reuse its space)
        att_t = s0in.tile([P, NCH * NTOK], BF)
        nc.sync.dma_start(out=att_t[:], in_=att_d[:, :])
        wo_t = s0in.tile([P, NCH * C], BF)
        nc.sync.dma_start(out=wo_t[:], in_=wo_d[:, :])
        xc_t = s0in.tile([P, NTB * C], BF)
        nc.sync.dma_start(out=xc_t[:], in_=xc_d[:, :])
        g1b = const.tile([P, C], FP)
        nc.sync.dma_start(out=g1b[:], in_=g1r_d[:, :])
        b1c = const.tile([P, NDF], FP)
        nc.sync.dma_start(out=b1c[:], in_=b1c_d[:, :])
        g2b = const.tile([P, C], FP)
        nc.sync.dma_start(out=g2b[:], in_=g2r_d[:, :])
        b2b = const.tile([P, C], FP)
        nc.sync.dma_start(out=b2b[:], in_=b2r_d[:, :])

        def rmsnorm_to(src, gb, out_t):
            # out = src * rsqrt(mean(src^2) + eps) * g, fused into 2 ACT + 2 DVE
            ssum = stats.tile([P, 1], FP, tag="ssum")
            sq = work.tile([P, C], FP, tag="sq")
            nc.scalar.activation(sq[:], src[:], AF.Square, accum_out=ssum[:])
            rstd = stats.tile([P, 1], FP, tag="rstd")
            nc.scalar.activation(rstd[:], ssum[:], AF.Sqrt,
                                 scale=1.0 / C, bias=eps_t[:])
            rinv = stats.tile([P, 1], FP, tag="rinv")
            nc.vector.reciprocal(rinv[:], rstd[:])
            nc.vector.scalar_tensor_tensor(
                out_t[:], src[:], rinv[:], gb[:],
                op0=mybir.AluOpType.mult, op1=mybir.AluOpType.mult)

        # ---- stage 0: o = attnT^T @ Wo; h = rmsnorm(x + bo + o) * g1; hT
        hbs = []                    # h in bf16 (residual base for r2)
        hb2s = []                   # h + b2 (fp32), precomputed for stage 3
        hts = [ht_pool.tile([P, NTOK], BF, tag=f"ht{c}", name=f"ht{c}")
               for c in range(NCH)]

        def stage0_mm(tb, o_ps):
            tiles = []
            for hst in range(0, C, 512):
                ps = o_ps.tile([P, 512], FP, tag="o", name="o")
                for c in range(NCH):
                    nc.tensor.matmul(
                        ps[:],
                        att_t[:, c * NTOK + tb * P:c * NTOK + (tb + 1) * P],
                        wo_t[:, c * C + hst:c * C + hst + 512],
                        start=(c == 0), stop=(c == NCH - 1))
                tiles.append(ps)
            return tiles

        def stage0_post(tb, tiles):
            r1 = work.tile([P, C], FP, tag="r1")
            for half, hst in enumerate(range(0, C, 512)):
                nc.vector.tensor_add(
                    r1[:, hst:hst + 512], tiles[half][:],
                    xc_t[:, tb * C + hst:tb * C + hst + 512])
            hb = h_pool.tile([P, C], BF, tag=f"h{tb}", name=f"h{tb}")
            rmsnorm_to(r1, g1b, hb)
            hbs.append(hb)
            hb2 = h_pool.tile([P, C], BF, tag=f"hb2{tb}", name=f"hb2{tb}")
            nc.vector.tensor_add(hb2[:], hb[:], b2b[:])
            hb2s.append(hb2)

        def stage0_transpose(tb, t_ps):
            for c in range(NCH):
                tp = t_ps.tile([P, P], BF, tag="tp", name="tp")
                nc.tensor.transpose(
                    tp[:], hbs[tb][:, c * P:(c + 1) * P], ident[:])
                nc.vector.tensor_copy(hts[c][:, tb * P:(tb + 1) * P], tp[:])

        with tc.tile_pool(name="o_ps", bufs=4, space="PSUM") as o_ps, \
             tc.tile_pool(name="t_ps", bufs=2, space="PSUM") as t_ps:
            pend = []
            for tb in range(NTB):
                tiles = stage0_mm(tb, o_ps)
                if pend:
                    stage0_transpose(pend[0], t_ps)
                    pend.pop()
                stage0_post(tb, tiles)
                pend.append(tb)
            for tb in pend:
                stage0_transpose(tb, t_ps)
        s0in.release()

        # ---- stage 2: aT = silu(W1^T @ hT + b1)  (W1 streamed, W2 prefetched)
        ats = []
        w2_pool = ctx.enter_context(tc.tile_pool(name="w2p", bufs=1))
        w2r = w2_pool.tile([P, C * DFF // P], BF)
        with tc.tile_pool(name="a_ps", bufs=6, space="PSUM") as a_ps, \
             tc.tile_pool(name="w1p", bufs=3) as w1p, \
             tc.tile_pool(name="sgp", bufs=3) as sgp:
            GW = NCH * 512          # per-g packed width in w1
            for g in range(NG):
                w1g = w1p.tile([P, GW], BF, tag="w1")
                nc.sync.dma_start(out=w1g[:], in_=w1_d[:, g * GW:(g + 1) * GW])
                # interleave the W2 prefetch with the W1 stream
                nc.sync.dma_start(
                    out=w2r[:, g * 4096:(g + 1) * 4096],
                    in_=w2_d[:, g * 4096:(g + 1) * 4096])
                aps = [a_ps.tile([P, NTOK], FP, tag="a", name="a")
                       for _ in range(4)]
                for c in range(NCH):
                    for u in range(4):
                        nc.tensor.matmul(
                            aps[u][:],
                            w1g[:, c * 512 + u * P:c * 512 + (u + 1) * P],
                            hts[c][:],
                            start=(c == 0), stop=(c == NCH - 1))
                for u in range(4):
                    d = 4 * g + u
                    sg = sgp.tile([P, NTOK], FP, tag="sg")
                    nc.scalar.activation(sg[:], aps[u][:], AF.Sigmoid,
                                         bias=b1c[:, d:d + 1], scale=1.0)
                    at_t = at_pool.tile([P, NTOK], BF, tag=f"at{d}")
                    nc.vector.scalar_tensor_tensor(
                        at_t[:], aps[u][:], b1c[:, d:d + 1], sg[:],
                        op0=mybir.AluOpType.add, op1=mybir.AluOpType.mult)
                    ats.append(at_t)

        # ---- stage 3: f = aT^T @ W2; out = rmsnorm(h + b2 + f) * g2
        with tc.tile_pool(name="f_ps", bufs=4, space="PSUM") as f_ps:
            for tb in range(NTB):
                tiles = []
                for hst in range(0, C, 512):
                    ps = f_ps.tile([P, 512], FP, tag="f", name="f")
                    for d in range(NDF):
                        nc.tensor.matmul(
                            ps[:],
                            ats[d][:, tb * P:(tb + 1) * P],
                            w2r[:, d * C + hst:d * C + hst + 512],
                            start=(d == 0), stop=(d == NDF - 1))
                    tiles.append(ps)
                r2 = work.tile([P, C], FP, tag="r2")
                for half, hst in enumerate(range(0, C, 512)):
                    nc.vector.tensor_add(
                        r2[:, hst:hst + 512], tiles[half][:],
                        hb2s[tb][:, hst:hst + 512])
                o_bf = work.tile([P, C], BF, tag="obf")
                rmsnorm_to(r2, g2b, o_bf)
                nc.sync.dma_start(
                    out=out_d[tb * P:(tb + 1) * P, :], in_=o_bf[:])
    nc.compile()
    return nc


def build_phase2_old(NTOK, C, DFF):
    NTB = NTOK // P
    NCH = C // P
    NDF = DFF // P
    NG = DFF // 512
    halves = [(st, min(512, C - st)) for st in range(0, C, 512)]
    NH = len(halves)            # <=512-wide chunks of the channel dim

    nc = bacc.Bacc("TRN2", debug=False)
    xc_d = nc.dram_tensor("xc", [NTOK, C], FP, kind="ExternalInput").ap()
    at_d = nc.dram_tensor("attnT", [C, NTOK], BF, kind="ExternalInput").ap()
    wo_d = nc.dram_tensor("wo", [C, C], BF, kind="ExternalInput").ap()
    w1_d = nc.dram_tensor("w1", [C, DFF], BF, kind="ExternalInput").ap()
    w2_d = nc.dram_tensor("w2", [DFF, C], BF, kind="ExternalInput").ap()
    g1_d = nc.dram_tensor("g1", [C], FP, kind="ExternalInput").ap()
    g2_d = nc.dram_tensor("g2", [C], FP, kind="ExternalInput").ap()
    b1_d = nc.dram_tensor("b1", [DFF], FP, kind="ExternalInput").ap()
    b2_d = nc.dram_tensor("b2", [C], FP, kind="ExternalInput").ap()
    out_d = nc.dram_tensor("out", [NTOK, C], FP, kind="ExternalOutput").ap()

    def bcast_rows(src_ap, cols):
        # DRAM vector [cols] -> [P, cols] (same row in every partition)
        return bass.AP(tensor=src_ap.tensor, offset=src_ap.offset,
                       ap=[[0, P], [1, cols]])

    def col_ap(src_ap, start):
        # DRAM vector slice [start:start+P] -> [P, 1] (one value per partition)
        return bass.AP(tensor=src_ap.tensor, offset=src_ap.offset + start,
                       ap=[[1, P], [0, 1]])

    with tile.TileContext(nc) as tc, ExitStack() as ctx:
        const = ctx.enter_context(tc.tile_pool(name="const", bufs=1))
        work = ctx.enter_context(tc.tile_pool(name="work", bufs=2))
        stats = ctx.enter_context(tc.tile_pool(name="stats", bufs=4))
        h_pool = ctx.enter_context(tc.tile_pool(name="hp", bufs=1))
        ht_pool = ctx.enter_context(tc.tile_pool(name="htp", bufs=1))
        at_pool = ctx.enter_context(tc.tile_pool(name="atp", bufs=1))

        ident = const.tile([P, P], FP)
        make_identity(nc, ident[:])
        eps_t = const.tile([P, 1], FP)
        nc.vector.memset(eps_t[:], EPS)
        g1b = const.tile([P, C], FP)
        nc.sync.dma_start(out=g1b[:], in_=bcast_rows(g1_d, C))
        g2b = const.tile([P, C], FP)
        nc.sync.dma_start(out=g2b[:], in_=bcast_rows(g2_d, C))
        b2b = const.tile([P, C], FP)
        nc.sync.dma_start(out=b2b[:], in_=bcast_rows(b2_d, C))
        b1s = []
        for d in range(NDF):
            t = const.tile([P, 1], FP, tag=f"b1_{d}")
            nc.sync.dma_start(out=t[:], in_=col_ap(b1_d, d * P))
            b1s.append(t)

        def rmsnorm(src, gb, out_tag):
            sq = work.tile([P, C], FP, tag="sq")
            ssum = stats.tile([P, 1], FP, tag="ssum")
            nc.scalar.activation(sq[:], src[:], AF.Square, accum_out=ssum[:])
            rstd = stats.tile([P, 1], FP, tag="rstd")
            nc.scalar.activation(rstd[:], ssum[:], AF.Sqrt,
                                 scale=1.0 / C, bias=eps_t[:])
            rinv = stats.tile([P, 1], FP, tag="rinv")
            nc.vector.reciprocal(rinv[:], rstd[:])
            out = work.tile([P, C], FP, tag=out_tag)
            nc.vector.tensor_scalar_mul(out[:], src[:], rinv[:])
            nc.vector.tensor_mul(out[:], out[:], gb[:])
            return out

        # ---- stage 0: o = attnT^T @ Wo; r1 = x + o; h = rmsnorm(r1)*g1
        hs = []
        with tc.tile_pool(name="o_ps", bufs=1, space="PSUM") as o_ps, \
             tc.tile_pool(name="wop", bufs=NCH) as wop, \
             tc.tile_pool(name="atsp", bufs=NCH) as atsp, \
             tc.tile_pool(name="xcp", bufs=1) as xcp:
            atts, wots = [], []
            for c in range(NCH):
                att = atsp.tile([P, NTOK], BF, tag="at", name="at")
                nc.sync.dma_start(out=att[:], in_=at_d[c * P:(c + 1) * P, :])
                wot = wop.tile([P, C], BF, tag="wo", name="wo")
                nc.sync.dma_start(out=wot[:], in_=wo_d[c * P:(c + 1) * P, :])
                atts.append(att)
                wots.append(wot)
            xcs = []
            for tb in range(NTB):
                t = xcp.tile([P, C], FP, tag=f"xc{tb}")
                nc.sync.dma_start(out=t[:], in_=xc_d[tb * P:(tb + 1) * P, :])
                xcs.append(t)
            o_tiles = [o_ps.tile([P, 512], FP, tag=f"ops{i}", name=f"ops{i}")
                       for i in range(NTB * NH)]
            for c in range(NCH):
                att = atts[c]
                wot = wots[c]
                for tb in range(NTB):
                    for half, (hst, hw) in enumerate(halves):
                        nc.tensor.matmul(
                            o_tiles[tb * NH + half][:, :hw],
                            att[:, tb * P:(tb + 1) * P],
                            wot[:, hst:hst + hw],
                            start=(c == 0), stop=(c == NCH - 1))
            for tb in range(NTB):
                r1 = work.tile([P, C], FP, tag="r1")
                for half, (hst, hw) in enumerate(halves):
                    nc.vector.tensor_add(
                        r1[:, hst:hst + hw],
                        o_tiles[tb * NH + half][:, :hw],
                        xcs[tb][:, hst:hst + hw])
                hn = rmsnorm(r1, g1b, "hn")
                h = h_pool.tile([P, C], FP, tag=f"h{tb}")
                nc.vector.tensor_copy(h[:], hn[:])
                hs.append(h)

        # ---- stage 1: hT
        hts = [ht_pool.tile([P, NTOK], BF, tag=f"ht{c}", name=f"ht{c}")
               for c in range(NCH)]
        with tc.tile_pool(name="t_ps", bufs=4, space="PSUM") as t_ps:
            for tb in range(NTB):
                for c in range(NCH):
                    tp = t_ps.tile([P, P], FP, tag="tp")
                    nc.tensor.transpose(
                        tp[:], hs[tb][:, c * P:(c + 1) * P], ident[:])
                    nc.vector.tensor_copy(hts[c][:, tb * P:(tb + 1) * P], tp[:])

        # ---- stage 2: aT = silu(W1^T @ h^T + b1)
        ats = []
        w2p = ctx.enter_context(tc.tile_pool(name="w2p", bufs=5))
        with tc.tile_pool(name="a_ps", bufs=8, space="PSUM") as a_ps, \
             tc.tile_pool(name="w1p", bufs=5) as w1p, \
             tc.tile_pool(name="sgp", bufs=3) as sgp:
            for g in range(NG):
                aps = [a_ps.tile([P, NTOK], FP, tag="a", name="a") for _ in range(4)]
                for c in range(NCH):
                    w1t = w1p.tile([P, 512], BF, tag="w1")
                    nc.sync.dma_start(
                        out=w1t[:],
                        in_=w1_d[c * P:(c + 1) * P, g * 512:(g + 1) * 512])
                    for u in range(4):
                        nc.tensor.matmul(
                            aps[u][:], w1t[:, u * P:(u + 1) * P],
                            hts[c][:],
                            start=(c == 0), stop=(c == NCH - 1))
                for u in range(4):
                    d = 4 * g + u
                    sg = sgp.tile([P, NTOK], FP, tag="sg")
                    nc.scalar.activation(sg[:], aps[u][:], AF.Sigmoid,
                                         bias=b1s[d][:], scale=1.0)
                    at_t = at_pool.tile([P, NTOK], BF, tag=f"at{d}")
                    # silu(z) for z = a + b1: (a + b1) * sigmoid(a + b1)
                    nc.vector.scalar_tensor_tensor(
                        at_t[:], aps[u][:], b1s[d][:], sg[:],
                        op0=mybir.AluOpType.add, op1=mybir.AluOpType.mult)
                    ats.append(at_t)

        # ---- stage 3: f = aT^T @ W2; r2 = h + b2 + f; out = rmsnorm(r2)*g2
        with tc.tile_pool(name="f_ps", bufs=1, space="PSUM") as f_ps:
            fts = [f_ps.tile([P, 512], FP, tag=f"f{i}", name=f"f{i}")
                   for i in range(NTB * NH)]
            for d in range(NDF):
                w2t = w2p.tile([P, C], BF, tag="w2")
                nc.sync.dma_start(out=w2t[:], in_=w2_d[d * P:(d + 1) * P, :])
                for tb in range(NTB):
                    for half, (hst, hw) in enumerate(halves):
                        nc.tensor.matmul(
                            fts[tb * NH + half][:, :hw],
                            ats[d][:, tb * P:(tb + 1) * P],
                            w2t[:, hst:hst + hw],
                            start=(d == 0), stop=(d == NDF - 1))
            for tb in range(NTB):
                hb = work.tile([P, C], FP, tag="hb")
                nc.vector.tensor_add(hb[:], hs[tb][:], b2b[:])
                r2 = work.tile([P, C], FP, tag="r2")
                for half, (hst, hw) in enumerate(halves):
                    nc.vector.tensor_add(
                        r2[:, hst:hst + hw],
                        fts[tb * NH + half][:, :hw],
                        hb[:, hst:hst + hw])
                o = rmsnorm(r2, g2b, "outt")
                nc.sync.dma_start(out=out_d[tb * P:(tb + 1) * P, :], in_=o[:])
    nc.compile()
    return nc


# --------------------------------------------------------------------------
# host orchestration
# --------------------------------------------------------------------------

_CACHE = {}


def _phase1(B, T, C, DH):
    key = ("p1", B, T, C, DH)
    if key not in _CACHE:
        _CACHE[key] = build_phase1(B, T, C, DH)
    return _CACHE[key]


def _phase2(NTOK, C, DFF):
    key = ("p2", NTOK, C, DFF)
    if key not in _CACHE:
        _CACHE[key] = build_phase2(NTOK, C, DFF)
    return _CACHE[key]


def _run(nc, in_maps):
    import os
    trace = bool(os.environ.get("KERNEL_TRACE"))
    res = run_bass_kernel_spmd(nc, in_maps, core_ids=list(range(N_CORES)),
                               trace=trace)
    LAST_EXEC_NS.append(res.exec_time_ns)
    return res.results


def kernel(x, Wq, Wk, Wv, Wo, bo, W1, b1, W2, b2, g1, g2):
    f32 = lambda a: np.ascontiguousarray(np.asarray(a), dtype=np.float32)
    x = f32(x)
    Wq, Wk, Wv, Wo, bo = f32(Wq), f32(Wk), f32(Wv), f32(Wo), f32(bo)
    W1, b1, W2, b2, g1, g2 = f32(W1), f32(b1), f32(W2), f32(b2), f32(g1), f32(g2)

    B, T, C = x.shape
    H, _, DH = Wq.shape
    HP = H // N_CORES           # heads per core (2)
    DA = DH + 1
    LAST_EXEC_NS.clear()

    # ---- phase 1
    nc1 = _phase1(B, T, C, DH)
    xT = np.ascontiguousarray(x.transpose(0, 2, 1)).astype(BF_NP)

    def pack_w(w, i):
        p = w[HP * i:HP * (i + 1)].transpose(1, 0, 2).reshape(C, HP * DH)
        return np.ascontiguousarray(
            p.reshape(C // 128, 128, HP * DH).transpose(1, 0, 2)
            .reshape(128, C)).astype(BF_NP)

    in1 = [{"xT": xT, "wq": pack_w(Wq, i), "wk": pack_w(Wk, i),
            "wv": pack_w(Wv, i)} for i in range(N_CORES)]
    res1 = _run(nc1, in1)

    attn = np.empty((B, T, C), np.float32)
    for i in range(N_CORES):
        ot = res1[i]["ot"].astype(np.float32)  # [B, HP, DA, T]
        o = ot[:, :, :DH, :]
        den = ot[:, :, DH, :]
        on = o / den[:, :, None, :]
        for hh in range(HP):
            hcol = (HP * i + hh) * DH
            attn[:, :, hcol:hcol + DH] = on[:, hh].transpose(0, 2, 1)

    # ---- phase 2
    NTOK = B * T // N_CORES
    DFF = W1.shape[1]
    NTB, NCH, NDF = NTOK // 128, C // 128, DFF // 128
    nc2 = _phase2(NTOK, C, DFF)
    xf = x.reshape(B * T, C) + bo             # fold bo into the residual
    af = attn.reshape(B * T, C)
    # partition-major packs: one big contiguous DMA per tensor on device
    wo_p = np.ascontiguousarray(
        Wo.reshape(NCH, 128, C).transpose(1, 0, 2).reshape(128, NCH * C)
    ).astype(BF_NP)
    w1_p = np.ascontiguousarray(
        W1.reshape(NCH, 128, DFF // 512, 512).transpose(1, 2, 0, 3)
        .reshape(128, C * DFF // 128)).astype(BF_NP)
    w2_p = np.ascontiguousarray(
        W2.reshape(NDF, 128, C).transpose(1, 0, 2).reshape(128, DFF * C // 128)
    ).astype(BF_NP)
    b1c = np.ascontiguousarray(b1.reshape(NDF, 128).T)
    g1r = np.ascontiguousarray(np.broadcast_to(g1, (128, C)))
    g2r = np.ascontiguousarray(np.broadcast_to(g2, (128, C)))
    b2r = np.ascontiguousarray(np.broadcast_to(b2, (128, C)))
    in2 = []
    for k in range(N_CORES):
        sl = slice(k * NTOK, (k + 1) * NTOK)
        att_p = np.ascontiguousarray(
            af[sl].T.reshape(NCH, 128, NTOK).transpose(1, 0, 2)
            .reshape(128, NCH * NTOK)).astype(BF_NP)
        xc_p = np.ascontiguousarray(
            xf[sl].reshape(NTB, 128, C).transpose(1, 0, 2)
            .reshape(128, NTB * C)).astype(BF_NP)
        in2.append({
            "att": att_p, "xc": xc_p, "wo": wo_p, "w1": w1_p, "w2": w2_p,
            "b1c": b1c, "g1r": g1r, "g2r": g2r, "b2r": b2r,
        })
    res2 = _run(nc2, in2)
    out = np.concatenate(
        [res2[k]["out"].astype(np.float32) for k in range(N_CORES)], axis=0)
    return out.reshape(B, T, C)

